# revision 1
# baseline (speedup 1.0000x reference)
"""Trainium2 Bass kernel for nn_GatedFeedForward (gated feed-forward with
feature attention).

Reference computation per batch b (B=8, N=4096, D=1024):
    VR = x @ Wvr.T + bvr ; VI = x @ Wvi.T + bvi
    V  = VR * tanh(softplus(VI))
    K  = x @ Wk.T + bk   ; Q  = x @ Wq.T + bq
    Kn = K / (||K||_col + 1e-5) ; Qn = Q / (||Q||_col + 1e-5)   (norm over N)
    A  = smu(Kn.T @ Qn)          # smu ~ leaky-relu(slope 0.25) for mu=1e6
    out = V @ A

Sharding: pure data-parallel over batch — 8 batches on 8 NeuronCores, one
batch per core, no collectives.

Math simplifications used by the kernel:
  * smu(x) = 0.5*((1+a)x + (1-a)x*erf(1e6*(1-a)x)) == 0.625x + 0.375|x|
    to within fp32 rounding for |x| >~ 5e-6 (erf saturates); the difference
    for tiny |x| is O(1e-6) absolute and vanishes in the D-sum.
  * leaky is positively homogeneous, so with rk=1/(||K||+1e-5), rq likewise:
        A = leaky((K.T Q) * rk[d] * rq[e]) = rk[d]*rq[e]*leaky(K.T Q)
    rk folds into A's rows (per-partition scale), rq folds into the final
    output tiles (free-dim broadcast multiply).

Kernel plan per core (all matmuls bf16 with fp32 PSUM accumulation):
  Pass 1 (per 512-row chunk of the sequence): K,Q = x@W, accumulate
      G += K_c.T Q_c in SBUF fp32, and column norms nk2/nq2 via an
      ones-vector matmul over squared K/Q tiles.
  Mid: rk/rq from norms; A' = rk[d] * (0.625 G + 0.375|G|) in bf16.
  Pass 2 (per chunk): VR,VI = x@W, V^T = VR * tanh(softplus(VI)),
      out_chunk = (V^T).T @ A' scaled by rq[e], DMA to DRAM.

Inputs are host-prepared: x[b] transposed to [D, N] bf16 (so the d
contraction dim lands on SBUF partitions), weights transposed to [D, D]
([in,out]) bf16. Biases are structurally zero for this problem (spec fill:
zeros); a host-side numpy fallback handles the never-expected nonzero case.
"""

import numpy as np
import ml_dtypes

import concourse.bass as bass
import concourse.tile as tile
from concourse import bacc, mybir
from concourse.bass import ts

F32 = mybir.dt.float32
BF16 = mybir.dt.bfloat16

B = 8
N_FULL = 4096
D_FULL = 1024
N_CORES = 8

P = 128  # SBUF partitions
NC = 512  # sequence chunk
EF = 512  # free-dim span per matmul / psum bank


def build_program(n=N_FULL, d=D_FULL):
    """Build the single-core SPMD Bass program for one [n, d] batch."""
    assert n % NC == 0 and d % P == 0
    n_chunks = n // NC
    n_sub = NC // P  # 128-row subtiles per chunk
    n_dblk = d // P  # contraction blocks
    ef = min(EF, d)
    n_ef = d // ef  # free-dim spans of the feature dim

    nc = bacc.Bacc("TRN2", target_bir_lowering=False, debug=False,
                   num_devices=N_CORES)
    xt = nc.dram_tensor("xt", [d, n], BF16, kind="ExternalInput")
    wvr = nc.dram_tensor("wvr", [d, d], BF16, kind="ExternalInput")
    wvi = nc.dram_tensor("wvi", [d, d], BF16, kind="ExternalInput")
    wk = nc.dram_tensor("wk", [d, d], BF16, kind="ExternalInput")
    wq = nc.dram_tensor("wq", [d, d], BF16, kind="ExternalInput")
    out_d = nc.dram_tensor("out", [n, d], F32, kind="ExternalOutput")

    with tile.TileContext(nc) as tc:
        with tc.tile_pool(name="const", bufs=1) as const_pool, \
             tc.tile_pool(name="weights", bufs=1) as w_pool, \
             tc.tile_pool(name="post", bufs=1) as post_pool, \
             tc.tile_pool(name="apost", bufs=1) as ap_pool:
            ones = const_pool.tile([P, 1], BF16, name="ones", tag="ones")
            nc.vector.memset(ones, 1.0)

            w_tiles = {}
            for wname, wdram in (("wk", wk), ("wq", wq), ("wvr", wvr), ("wvi", wvi)):
                tl = []
                for db in range(n_dblk):
                    t = w_pool.tile([P, d], BF16, name=f"{wname}{db}", tag=f"{wname}{db}")
                    nc.sync.dma_start(out=t, in_=wdram[ts(db, P), :])
                    tl.append(t)
                w_tiles[wname] = tl

            # ---------------- Pass 1: K, Q -> G, norms ----------------
            with tc.tile_pool(name="xt1", bufs=2) as xt_pool, \
                 tc.tile_pool(name="kq_sb", bufs=1) as kq_sb_pool, \
                 tc.tile_pool(name="sq_sb", bufs=1) as sq_pool, \
                 tc.tile_pool(name="gacc", bufs=1) as g_pool, \
                 tc.tile_pool(name="nrm", bufs=1) as nrm_pool:

                g_tiles = [
                    g_pool.tile([P, d], F32, name=f"g{db}", tag=f"g{db}")
                    for db in range(n_dblk)
                ]
                nk2 = nrm_pool.tile([1, d], F32, name="nk2", tag="nk2")
                nq2 = nrm_pool.tile([1, d], F32, name="nq2", tag="nq2")

                with tc.tile_pool(name="kq_ps", bufs=5, space="PSUM") as kq_ps, \
                     tc.tile_pool(name="g_ps", bufs=2, space="PSUM") as g_ps, \
                     tc.tile_pool(name="nrm_ps", bufs=1, space="PSUM") as nrm_ps:
                    for c in range(n_chunks):
                        xts = []
                        for db in range(n_dblk):
                            t = xt_pool.tile([P, NC], BF16, name=f"x1_{db}", tag=f"x1_{db}")
                            nc.sync.dma_start(out=t, in_=xt[ts(db, P), ts(c, NC)])
                            xts.append(t)

                        k_sb, q_sb, k_sq, q_sq = [], [], [], []
                        for s in range(n_sub):
                            kt = kq_sb_pool.tile([P, d], BF16, name=f"k{s}", tag=f"k{s}")
                            qt = kq_sb_pool.tile([P, d], BF16, name=f"q{s}", tag=f"q{s}")
                            kst = sq_pool.tile([P, d], BF16, name=f"ksq{s}", tag=f"ksq{s}")
                            qst = sq_pool.tile([P, d], BF16, name=f"qsq{s}", tag=f"qsq{s}")
                            for e in range(n_ef):
                                pk = kq_ps.tile([P, ef], F32, name="pk", tag="kqps")
                                pq = kq_ps.tile([P, ef], F32, name="pq", tag="kqps")
                                for db in range(n_dblk):
                                    st = dict(start=(db == 0), stop=(db == n_dblk - 1))
                                    lhsT = xts[db][:, ts(s, P)]
                                    nc.tensor.matmul(pk, lhsT=lhsT, rhs=w_tiles["wk"][db][:, ts(e, ef)], **st)
                                    nc.tensor.matmul(pq, lhsT=lhsT, rhs=w_tiles["wq"][db][:, ts(e, ef)], **st)
                                nc.vector.tensor_copy(out=kt[:, ts(e, ef)], in_=pk)
                                nc.vector.tensor_copy(out=qt[:, ts(e, ef)], in_=pq)
                                nc.scalar.activation(out=kst[:, ts(e, ef)], in_=pk,
                                                     func=mybir.ActivationFunctionType.Square)
                                nc.scalar.activation(out=qst[:, ts(e, ef)], in_=pq,
                                                     func=mybir.ActivationFunctionType.Square)
                            k_sb.append(kt)
                            q_sb.append(qt)
                            k_sq.append(kst)
                            q_sq.append(qst)

                        # G[d, e] += sum_n K[n, d] * Q[n, e]
                        for db in range(n_dblk):
                            for e in range(n_ef):
                                gp = g_ps.tile([P, ef], F32, name="gp", tag="gps")
                                for s in range(n_sub):
                                    nc.tensor.matmul(gp, lhsT=k_sb[s][:, ts(db, P)],
                                                     rhs=q_sb[s][:, ts(e, ef)],
                                                     start=(s == 0), stop=(s == n_sub - 1))
                                if c == 0:
                                    nc.vector.tensor_copy(out=g_tiles[db][:, ts(e, ef)], in_=gp)
                                else:
                                    nc.vector.tensor_add(out=g_tiles[db][:, ts(e, ef)],
                                                         in0=g_tiles[db][:, ts(e, ef)], in1=gp)

                        # column norms: nk2[e] += sum_n K[n, e]^2
                        for sq_list, acc in ((k_sq, nk2), (q_sq, nq2)):
                            for e in range(n_ef):
                                pn = nrm_ps.tile([1, ef], F32, name="pn", tag="nrmps")
                                for s in range(n_sub):
                                    nc.tensor.matmul(pn, lhsT=ones, rhs=sq_list[s][:, ts(e, ef)],
                                                     start=(s == 0), stop=(s == n_sub - 1))
                                if c == 0:
                                    nc.vector.tensor_copy(out=acc[:, ts(e, ef)], in_=pn)
                                else:
                                    nc.vector.tensor_add(out=acc[:, ts(e, ef)],
                                                         in0=acc[:, ts(e, ef)], in1=pn)

                # ---------------- Mid: rk, rq, A' ----------------
                nk = post_pool.tile([1, d], F32, name="nk", tag="nk")
                nq = post_pool.tile([1, d], F32, name="nq", tag="nq")
                nc.scalar.activation(out=nk, in_=nk2, func=mybir.ActivationFunctionType.Sqrt)
                nc.scalar.activation(out=nq, in_=nq2, func=mybir.ActivationFunctionType.Sqrt)
                nc.vector.tensor_scalar_add(out=nk, in0=nk, scalar1=1e-5)
                nc.vector.tensor_scalar_add(out=nq, in0=nq, scalar1=1e-5)
                nc.vector.reciprocal(out=nk, in_=nk)
                nc.vector.reciprocal(out=nq, in_=nq)

                # rk as per-partition columns [P, n_dblk] (via 1-deep matmuls
                # that load each 128-wide rk slice as stationary weights), and
                # rq broadcast across partitions [P, d] (rank-1 outer product
                # with a ones row). Internal-DRAM round trips don't load on
                # this runtime, so both transposes stay on the PE.
                one11 = post_pool.tile([1, 1], F32, name="one11", tag="one11")
                nc.vector.memset(one11, 1.0)
                ones_row = post_pool.tile([1, P], F32, name="ones_row", tag="ones_row")
                nc.vector.memset(ones_row, 1.0)

                rk_col = post_pool.tile([P, n_dblk], F32, name="rk_col", tag="rk_col")
                rq_bc = post_pool.tile([P, d], F32, name="rq_bc", tag="rq_bc")
                with tc.tile_pool(name="misc_ps", bufs=2, space="PSUM") as misc_ps:
                    for db in range(n_dblk):
                        pt = misc_ps.tile([P, 1], F32, name="pt", tag="miscps")
                        nc.tensor.matmul(pt, lhsT=nk[0:1, ts(db, P)], rhs=one11,
                                         start=True, stop=True)
                        nc.vector.tensor_copy(out=rk_col[:, db:db + 1], in_=pt)
                    for e in range(n_ef):
                        pb = misc_ps.tile([P, ef], F32, name="pb", tag="miscps_b")
                        nc.tensor.matmul(pb, lhsT=ones_row, rhs=nq[0:1, ts(e, ef)],
                                         start=True, stop=True)
                        nc.vector.tensor_copy(out=rq_bc[:, ts(e, ef)], in_=pb)

                rk625 = post_pool.tile([P, n_dblk], F32, name="rk625", tag="rk625")
                rk375 = post_pool.tile([P, n_dblk], F32, name="rk375", tag="rk375")
                nc.vector.tensor_scalar_mul(out=rk625, in0=rk_col, scalar1=0.625)
                nc.vector.tensor_scalar_mul(out=rk375, in0=rk_col, scalar1=0.375)

                # A'[d, e] = rk[d] * (0.625 G + 0.375 |G|), bf16
                a_tiles = []
                with tc.tile_pool(name="tabs", bufs=2) as tabs_pool:
                    for db in range(n_dblk):
                        at = ap_pool.tile([P, d], BF16, name=f"a{db}", tag=f"a{db}")
                        tabs = tabs_pool.tile([P, d], F32, name="tabs", tag="tabs")
                        nc.scalar.activation(out=tabs, in_=g_tiles[db],
                                             func=mybir.ActivationFunctionType.Abs,
                                             scale=rk375[:, db:db + 1])
                        nc.vector.scalar_tensor_tensor(out=at, in0=g_tiles[db],
                                                       scalar=rk625[:, db:db + 1], in1=tabs,
                                                       op0=mybir.AluOpType.mult,
                                                       op1=mybir.AluOpType.add)
                        a_tiles.append(at)

            # ---------------- Pass 2: V, output ----------------
            with tc.tile_pool(name="xt2", bufs=2) as xt2_pool, \
                 tc.tile_pool(name="vt", bufs=2) as vt_pool, \
                 tc.tile_pool(name="gate", bufs=2) as gate_pool, \
                 tc.tile_pool(name="osb", bufs=3) as osb_pool, \
                 tc.tile_pool(name="vrvi_ps", bufs=4, space="PSUM") as vrvi_ps, \
                 tc.tile_pool(name="out_ps", bufs=3, space="PSUM") as out_ps:
                for c in range(n_chunks):
                    xts = []
                    for db in range(n_dblk):
                        t = xt2_pool.tile([P, NC], BF16, name=f"x2_{db}", tag=f"x2_{db}")
                        nc.sync.dma_start(out=t, in_=xt[ts(db, P), ts(c, NC)])
                        xts.append(t)

                    # V^T[e, n] tiles, e on partitions
                    vts = []
                    for eb in range(n_dblk):
                        pvr = vrvi_ps.tile([P, NC], F32, name="pvr", tag="vrvi")
                        pvi = vrvi_ps.tile([P, NC], F32, name="pvi", tag="vrvi")
                        for db in range(n_dblk):
                            st = dict(start=(db == 0), stop=(db == n_dblk - 1))
                            nc.tensor.matmul(pvr, lhsT=w_tiles["wvr"][db][:, ts(eb, P)], rhs=xts[db], **st)
                            nc.tensor.matmul(pvi, lhsT=w_tiles["wvi"][db][:, ts(eb, P)], rhs=xts[db], **st)
                        # gate = tanh(softplus(vi)); with s = sigmoid(vi),
                        # m = (1-s)^2:  gate = (1-m)/(1+m) = 2/(1+m) - 1.
                        # (softplus is not in any TRN2 activation table.)
                        sg = gate_pool.tile([P, NC], F32, name="sg", tag="sg")
                        nc.scalar.activation(out=sg, in_=pvi, func=mybir.ActivationFunctionType.Sigmoid)
                        m = gate_pool.tile([P, NC], F32, name="m", tag="m")
                        nc.scalar.activation(out=m, in_=sg, func=mybir.ActivationFunctionType.Square,
                                             scale=-1.0, bias=1.0)
                        nc.vector.tensor_scalar_add(out=m, in0=m, scalar1=1.0)
                        nc.vector.reciprocal(out=m, in_=m)  # r = 1/(1+m)
                        # V = VR * (2r - 1)
                        v2 = gate_pool.tile([P, NC], F32, name="v2", tag="v2")
                        nc.vector.scalar_tensor_tensor(out=v2, in0=pvr, scalar=2.0, in1=m,
                                                       op0=mybir.AluOpType.mult,
                                                       op1=mybir.AluOpType.mult)
                        vt = vt_pool.tile([P, NC], BF16, name=f"vt{eb}", tag=f"vt{eb}")
                        nc.vector.tensor_tensor(out=vt, in0=v2, in1=pvr,
                                                op=mybir.AluOpType.subtract)
                        vts.append(vt)

                    # out[n, e] = rq[e] * sum_d V[n, d] A'[d, e]
                    for s in range(n_sub):
                        for e in range(n_ef):
                            po = out_ps.tile([P, ef], F32, name="po", tag="ops")
                            for db in range(n_dblk):
                                nc.tensor.matmul(po, lhsT=vts[db][:, ts(s, P)],
                                                 rhs=a_tiles[db][:, ts(e, ef)],
                                                 start=(db == 0), stop=(db == n_dblk - 1))
                            ot = osb_pool.tile([P, ef], F32, name="ot", tag="osb")
                            nc.vector.tensor_mul(out=ot, in0=po, in1=rq_bc[:, ts(e, ef)])
                            nc.sync.dma_start(
                                out=out_d[c * NC + s * P:c * NC + (s + 1) * P, ts(e, ef)],
                                in_=ot)
    nc.compile()
    return nc


_PROGRAM_CACHE = {}


def _get_program(n, d):
    key = (n, d)
    if key not in _PROGRAM_CACHE:
        _PROGRAM_CACHE[key] = build_program(n, d)
    return _PROGRAM_CACHE[key]


def _numpy_reference(x, Wvr, bvr, Wvi, bvi, Wk, bk, Wq, bq):
    """Slow fp32 fallback (never expected to run: biases are zeros)."""
    out = np.empty_like(x)
    for b in range(x.shape[0]):
        xb = x[b].astype(np.float64)
        vr = xb @ Wvr.T.astype(np.float64) + bvr
        vi = xb @ Wvi.T.astype(np.float64) + bvi
        v = vr * np.tanh(np.logaddexp(0.0, vi))
        k = xb @ Wk.T.astype(np.float64) + bk
        q = xb @ Wq.T.astype(np.float64) + bq
        kn = k / (np.linalg.norm(k, axis=0, keepdims=True) + 1e-5)
        qn = q / (np.linalg.norm(q, axis=0, keepdims=True) + 1e-5)
        g = kn.T @ qn
        a = 0.625 * g + 0.375 * np.abs(g)
        out[b] = (v @ a).astype(np.float32)
    return out


def kernel(_run_kwargs=None, **inputs):
    run_kwargs = _run_kwargs or {}
    x = np.asarray(inputs["x"], dtype=np.float32)
    Wvr = np.asarray(inputs["Wvr"], dtype=np.float32)
    Wvi = np.asarray(inputs["Wvi"], dtype=np.float32)
    Wk = np.asarray(inputs["Wk"], dtype=np.float32)
    Wq = np.asarray(inputs["Wq"], dtype=np.float32)
    bvr, bvi = np.asarray(inputs["bvr"]), np.asarray(inputs["bvi"])
    bk, bq = np.asarray(inputs["bk"]), np.asarray(inputs["bq"])

    if any(np.any(b != 0) for b in (bvr, bvi, bk, bq)):
        return _numpy_reference(x, Wvr, bvr, Wvi, bvi, Wk, bk, Wq, bq)

    b, n, d = x.shape
    assert b == B and n == N_FULL and d == D_FULL, (b, n, d)

    bf16 = ml_dtypes.bfloat16
    wvr_t = np.ascontiguousarray(Wvr.T).astype(bf16)
    wvi_t = np.ascontiguousarray(Wvi.T).astype(bf16)
    wk_t = np.ascontiguousarray(Wk.T).astype(bf16)
    wq_t = np.ascontiguousarray(Wq.T).astype(bf16)

    in_maps = []
    for i in range(N_CORES):
        in_maps.append({
            "xt": np.ascontiguousarray(x[i].T).astype(bf16),
            "wvr": wvr_t, "wvi": wvi_t, "wk": wk_t, "wq": wq_t,
        })

    nc = _get_program(n, d)
    from concourse.bass_utils import run_bass_kernel_spmd
    res = run_bass_kernel_spmd(nc, in_maps, core_ids=list(range(N_CORES)), **run_kwargs)
    out = np.stack([res.results[i]["out"] for i in range(N_CORES)], axis=0)
    if run_kwargs:
        kernel.last_results = res
    return out



# revision 9
# speedup vs baseline: 1.4348x; 1.4348x over previous
"""Trainium2 Bass kernel for nn_GatedFeedForward (gated feed-forward with
feature attention).

Reference computation per batch b (B=8, N=4096, D=1024):
    VR = x @ Wvr.T ; VI = x @ Wvi.T            (biases are zero)
    V  = VR * tanh(softplus(VI))
    K  = x @ Wk.T  ; Q  = x @ Wq.T
    Kn = K / (||K||_col + 1e-5) ; Qn = Q / (||Q||_col + 1e-5)   (norm over N)
    A  = smu(Kn.T @ Qn)     # == leaky-relu slope 0.25 == 0.625x + 0.375|x|
    out = V @ A
Sharding: pure data-parallel over batch — one batch per NeuronCore.

Key algebraic restructure: with S = X^T X (D x D, one N-contraction),
    K^T Q        = WkT^T S WqT          (WkT = Wk.T, [in,out])
    ||K_d||^2    = colsum(WkT * (S WkT))
    ||Q_e||^2    = colsum(WqT * (S WqT))
so the K/Q path costs one N*D^2 matmul (S) plus three D^3 matmuls
(Tk = S WkT, G = Tk^T WqT, Uq = S WqT) instead of two N*D^2 (K, Q) plus
one N*D^2 (K^T Q): 2.4x less PE time on that path. leaky's positive
homogeneity folds rk into A's rows and rq into the output tiles.

The gate tanh(softplus(x)) is evaluated as c0 + c1*tanh(a1 x + b1)
+ c2*tanh(a2 x + b2) (max abs err 3.9e-3): both ops hit the resident
tanh activation table — no table switches, no slow DVE reciprocal.

Schedule per core (all matmuls bf16 with fp32 PSUM):
  Pass 1: S accumulated directly in PSUM across all 8 sequence chunks,
      in two column-half phases of 8 banks each; cast to bf16 at phase end.
  Mid:    Tk (+nk2 via ones-matmuls), Uq (+nq2), G -> A' from PSUM;
      norm scalars overlap the Uq matmuls on ACT/DVE.
  Pass 2: VRVI(c) / out(c) software-pipelined with lag 1 so the PE never
      waits on the gate chain. Output stored bf16, upcast on host.
"""

import numpy as np
import ml_dtypes

import concourse.bass as bass
import concourse.tile as tile
from concourse import bacc, mybir
from concourse.bass import ts

F32 = mybir.dt.float32
BF16 = mybir.dt.bfloat16
AF = mybir.ActivationFunctionType
ALU = mybir.AluOpType

B = 8
N_FULL = 4096
D_FULL = 1024
N_CORES = 8

P = 128  # SBUF partitions
NC = 512  # sequence chunk
EF = 512  # free-dim span per matmul / psum bank

# tanh-sum fit of tanh(softplus(x)), max |err| 3.9e-3 on [-12, 12]
GC0, GC1, GA1, GB1 = 0.50022747, 0.32785149, 0.8261997, -0.02962021
GC2, GA2, GB2 = 0.17216236, 0.57575332, 0.75023909


def build_program(n=N_FULL, d=D_FULL):
    """Build the single-core SPMD Bass program for one [n, d] batch."""
    assert n % NC == 0 and d % P == 0
    n_chunks = n // NC
    n_sub = NC // P  # 128-row subtiles per chunk
    n_dblk = d // P  # feature blocks
    ef = min(EF, d)
    n_ef = d // ef

    nc = bacc.Bacc("TRN2", target_bir_lowering=False, debug=False,
                   num_devices=N_CORES)
    xn = nc.dram_tensor("xn", [n, d], BF16, kind="ExternalInput")
    xt = nc.dram_tensor("xt", [d, n], BF16, kind="ExternalInput")
    wvr = nc.dram_tensor("wvr", [d, d], BF16, kind="ExternalInput")
    wvi = nc.dram_tensor("wvi", [d, d], BF16, kind="ExternalInput")
    wk = nc.dram_tensor("wk", [d, d], BF16, kind="ExternalInput")
    wq = nc.dram_tensor("wq", [d, d], BF16, kind="ExternalInput")
    out_d = nc.dram_tensor("out", [n, d], BF16, kind="ExternalOutput")

    with tile.TileContext(nc) as tc:
        with tc.tile_pool(name="const", bufs=1) as const_pool, \
             tc.tile_pool(name="w", bufs=1) as w_pool, \
             tc.tile_pool(name="sb", bufs=1) as sb_pool, \
             tc.tile_pool(name="tkb", bufs=1) as tkb_pool, \
             tc.tile_pool(name="post", bufs=1) as post_pool, \
             tc.tile_pool(name="apost", bufs=1) as ap_pool:
            ones32 = const_pool.tile([P, 1], F32, name="ones32", tag="ones32")
            nc.vector.memset(ones32, 1.0)
            one11 = const_pool.tile([1, 1], F32, name="one11", tag="one11")
            nc.vector.memset(one11, 1.0)
            ones_row = const_pool.tile([1, P], F32, name="ones_row", tag="onesr")
            nc.vector.memset(ones_row, 1.0)
            gb1 = const_pool.tile([P, 1], F32, name="gb1", tag="gb1")
            nc.vector.memset(gb1, GB1)
            gb2 = const_pool.tile([P, 1], F32, name="gb2", tag="gb2")
            nc.vector.memset(gb2, GB2)

            w_tiles = {}
            for wname, wdram in (("wk", wk), ("wq", wq), ("wvr", wvr), ("wvi", wvi)):
                w_tiles[wname] = [
                    w_pool.tile([P, d], BF16, name=f"{wname}{db}", tag=f"{wname}{db}")
                    for db in range(n_dblk)
                ]

            sb_tiles = [sb_pool.tile([P, d], BF16, name=f"s{ib}", tag=f"s{ib}")
                        for ib in range(n_dblk)]
            tkb_tiles = [tkb_pool.tile([P, d], BF16, name=f"tk{ib}", tag=f"tk{ib}")
                         for ib in range(n_dblk)]
            a_tiles = [ap_pool.tile([P, d], BF16, name=f"a{db}", tag=f"a{db}")
                       for db in range(n_dblk)]

            # ---------------- Pass 1: S = X^T X, PSUM-resident ----------------
            with tc.tile_pool(name="xn1", bufs=3) as xn_pool, \
                 tc.tile_pool(name="s_ps", bufs=1, space="PSUM") as s_ps:

                def dma_xn(e, c):
                    tiles = []
                    for s in range(n_sub):
                        t = xn_pool.tile([P, d], BF16, name=f"xn{s}", tag=f"xn{s}")
                        nc.sync.dma_start(out=t, in_=xn[c * NC + s * P:c * NC + (s + 1) * P, :])
                        tiles.append(t)
                    return tiles

                seq = [(e, c) for e in range(n_ef) for c in range(n_chunks)]
                xn_cache = {seq[0]: dma_xn(*seq[0])}
                first = True
                for idx, (e, c) in enumerate(seq):
                    if first:
                        # weight DMAs issued after the first x chunk
                        for wname, wdram in (("wk", wk), ("wq", wq),
                                             ("wvr", wvr), ("wvi", wvi)):
                            for db in range(n_dblk):
                                nc.sync.dma_start(out=w_tiles[wname][db],
                                                  in_=wdram[ts(db, P), :])
                        first = False
                    if idx + 1 < len(seq):
                        xn_cache[seq[idx + 1]] = dma_xn(*seq[idx + 1])
                    if c == 0:
                        ps_list = [s_ps.tile([P, ef], F32, name=f"sps{ib}", tag=f"sps{ib}")
                                   for ib in range(n_dblk)]
                    xns = xn_cache.pop((e, c))
                    for s in range(n_sub):
                        for ib in range(n_dblk):
                            nc.tensor.matmul(ps_list[ib], lhsT=xns[s][:, ts(ib, P)],
                                             rhs=xns[s][:, ts(e, ef)],
                                             start=(c == 0 and s == 0),
                                             stop=(c == n_chunks - 1 and s == n_sub - 1))
                    if c == n_chunks - 1:
                        for ib in range(n_dblk):
                            nc.vector.tensor_copy(out=sb_tiles[ib][:, ts(e, ef)],
                                                  in_=ps_list[ib])

            # ---------------- pass-2 SBUF pools (open early for prefetch) ------
            nk = post_pool.tile([1, d], F32, name="nk", tag="nk")
            nq = post_pool.tile([1, d], F32, name="nq", tag="nq")
            rk_col = post_pool.tile([P, n_dblk], F32, name="rk_col", tag="rk_col")
            rq_bc = post_pool.tile([P, d], F32, name="rq_bc", tag="rq_bc")
            rk625 = post_pool.tile([P, n_dblk], F32, name="rk625", tag="rk625")
            rk375 = post_pool.tile([P, n_dblk], F32, name="rk375", tag="rk375")

            with tc.tile_pool(name="xt2", bufs=2) as xt2_pool, \
                 tc.tile_pool(name="vt", bufs=2) as vt_pool, \
                 tc.tile_pool(name="gate", bufs=2) as gate_pool, \
                 tc.tile_pool(name="wtmp", bufs=4) as wtmp_pool, \
                 tc.tile_pool(name="osb", bufs=3) as osb_pool:

                def dma_xt(c):
                    tiles = []
                    for db in range(n_dblk):
                        t = xt2_pool.tile([P, NC], BF16, name=f"x2_{db}", tag=f"x2_{db}")
                        nc.sync.dma_start(out=t, in_=xt[ts(db, P), ts(c, NC)])
                        tiles.append(t)
                    return tiles

                xts_cache = {0: dma_xt(0), 1: dma_xt(1)}

                # ---------------- Mid: Tk, Uq, G, norms, A' ----------------
                with tc.tile_pool(name="mid_ps", bufs=4, space="PSUM") as mid_ps, \
                     tc.tile_pool(name="nrm_ps", bufs=1, space="PSUM") as nrm_ps:

                    # Tk = S @ WkT ; nk2 = colsum(WkT * Tk)
                    # software-pipeline the ones-matmuls one group behind so the
                    # PE never waits on the DVE mult
                    pend = []  # (ones_mm_args) delayed by one group

                    def flush_pend():
                        while pend:
                            pn, tmp, st = pend.pop(0)
                            nc.tensor.matmul(pn, lhsT=ones32, rhs=tmp, **st)

                    nk2_ps = {e: nrm_ps.tile([1, ef], F32, name=f"nk2{e}", tag=f"nrm{e}")
                              for e in range(n_ef)}
                    for e in range(n_ef):
                        for ib in range(n_dblk):
                            pt = mid_ps.tile([P, ef], F32, name="pt", tag="midps")
                            for jb in range(n_dblk):
                                nc.tensor.matmul(pt, lhsT=sb_tiles[jb][:, ts(ib, P)],
                                                 rhs=w_tiles["wk"][jb][:, ts(e, ef)],
                                                 start=(jb == 0), stop=(jb == n_dblk - 1))
                            nc.vector.tensor_copy(out=tkb_tiles[ib][:, ts(e, ef)], in_=pt)
                            tmp = wtmp_pool.tile([P, ef], F32, name="tmp", tag="wtmp")
                            nc.vector.tensor_mul(out=tmp, in0=pt,
                                                 in1=w_tiles["wk"][ib][:, ts(e, ef)])
                            flush_pend()
                            pend.append((nk2_ps[e], tmp,
                                         dict(start=(ib == 0), stop=(ib == n_dblk - 1))))
                    flush_pend()

                    # rk chain on ACT/DVE (overlaps the Uq matmuls below)
                    for e in range(n_ef):
                        nc.scalar.activation(out=nk[0:1, ts(e, ef)], in_=nk2_ps[e],
                                             func=AF.Sqrt)
                    nc.vector.tensor_scalar_add(out=nk, in0=nk, scalar1=1e-5)
                    nc.vector.reciprocal(out=nk, in_=nk)

                    # Uq = S @ WqT ; nq2 = colsum(WqT * Uq)
                    nq2_ps = {}
                    for e in range(n_ef):
                        # reuses nk2's bank; allocated after the sqrt reads above
                        nq2_ps[e] = nrm_ps.tile([1, ef], F32, name=f"nq2{e}", tag=f"nrm{e}")
                        for ib in range(n_dblk):
                            pt = mid_ps.tile([P, ef], F32, name="pu", tag="midps")
                            for jb in range(n_dblk):
                                nc.tensor.matmul(pt, lhsT=sb_tiles[jb][:, ts(ib, P)],
                                                 rhs=w_tiles["wq"][jb][:, ts(e, ef)],
                                                 start=(jb == 0), stop=(jb == n_dblk - 1))
                            tmp = wtmp_pool.tile([P, ef], F32, name="tmq", tag="wtmp")
                            nc.vector.tensor_mul(out=tmp, in0=pt,
                                                 in1=w_tiles["wq"][ib][:, ts(e, ef)])
                            flush_pend()
                            pend.append((nq2_ps[e], tmp,
                                         dict(start=(ib == 0), stop=(ib == n_dblk - 1))))
                        if e == 0:
                            # rk transpose to per-partition column (tiny PE matmuls
                            # interleaved between Uq groups)
                            for db in range(n_dblk):
                                pm = mid_ps.tile([P, ef], F32, name="pm", tag="midps")
                                nc.tensor.matmul(pm[:, 0:1], lhsT=nk[0:1, ts(db, P)],
                                                 rhs=one11, start=True, stop=True)
                                nc.vector.tensor_copy(out=rk_col[:, db:db + 1],
                                                      in_=pm[:, 0:1])
                            nc.vector.tensor_scalar_mul(out=rk625, in0=rk_col,
                                                        scalar1=0.625)
                            nc.vector.tensor_scalar_mul(out=rk375, in0=rk_col,
                                                        scalar1=0.375)
                    flush_pend()

                    # rq chain
                    for e in range(n_ef):
                        nc.scalar.activation(out=nq[0:1, ts(e, ef)], in_=nq2_ps[e],
                                             func=AF.Sqrt)
                    nc.vector.tensor_scalar_add(out=nq, in0=nq, scalar1=1e-5)
                    nc.vector.reciprocal(out=nq, in_=nq)
                    for e in range(n_ef):
                        pb = mid_ps.tile([P, ef], F32, name="pb", tag="midps")
                        nc.tensor.matmul(pb, lhsT=ones_row, rhs=nq[0:1, ts(e, ef)],
                                         start=True, stop=True)
                        nc.vector.tensor_copy(out=rq_bc[:, ts(e, ef)], in_=pb)

                    # G = Tk^T @ WqT ; A' = rk * (0.625 G + 0.375 |G|) from PSUM
                    for eb in range(n_dblk):
                        for e in range(n_ef):
                            pg = mid_ps.tile([P, ef], F32, name="pg", tag="midps")
                            for db in range(n_dblk):
                                nc.tensor.matmul(pg, lhsT=tkb_tiles[db][:, ts(eb, P)],
                                                 rhs=w_tiles["wq"][db][:, ts(e, ef)],
                                                 start=(db == 0), stop=(db == n_dblk - 1))
                            tabs = wtmp_pool.tile([P, ef], F32, name="tabs", tag="wtmp")
                            nc.scalar.activation(out=tabs, in_=pg, func=AF.Abs,
                                                 scale=rk375[:, eb:eb + 1])
                            nc.vector.scalar_tensor_tensor(
                                out=a_tiles[eb][:, ts(e, ef)], in0=pg,
                                scalar=rk625[:, eb:eb + 1], in1=tabs,
                                op0=ALU.mult, op1=ALU.add)

                # ---------------- Pass 2: VRVI / out pipeline ----------------
                with tc.tile_pool(name="vrvi_ps", bufs=4, space="PSUM") as vrvi_ps, \
                     tc.tile_pool(name="out_ps", bufs=3, space="PSUM") as out_ps:

                    def issue_vrvi(c, xts):
                        vts = []
                        for eb in range(n_dblk):
                            pvr = vrvi_ps.tile([P, NC], F32, name="pvr", tag="vrvi")
                            pvi = vrvi_ps.tile([P, NC], F32, name="pvi", tag="vrvi")
                            for db in range(n_dblk):
                                st = dict(start=(db == 0), stop=(db == n_dblk - 1))
                                nc.tensor.matmul(pvr, lhsT=w_tiles["wvr"][db][:, ts(eb, P)],
                                                 rhs=xts[db], **st)
                                nc.tensor.matmul(pvi, lhsT=w_tiles["wvi"][db][:, ts(eb, P)],
                                                 rhs=xts[db], **st)
                            # gate = c0 + c1*tanh(a1 x + b1) + c2*tanh(a2 x + b2)
                            t1 = gate_pool.tile([P, NC], F32, name="t1", tag="t1")
                            nc.scalar.activation(out=t1, in_=pvi, func=AF.Tanh,
                                                 scale=GA1, bias=gb1)
                            t2 = gate_pool.tile([P, NC], F32, name="t2", tag="t2")
                            nc.scalar.activation(out=t2, in_=pvi, func=AF.Tanh,
                                                 scale=GA2, bias=gb2)
                            g1 = gate_pool.tile([P, NC], F32, name="g1", tag="g1")
                            nc.vector.tensor_scalar_mul(out=g1, in0=t2, scalar1=GC2 / GC1)
                            g2 = gate_pool.tile([P, NC], F32, name="g2", tag="g2")
                            nc.vector.tensor_add(out=g2, in0=t1, in1=g1)
                            g3 = gate_pool.tile([P, NC], F32, name="g3", tag="g3")
                            nc.vector.tensor_scalar_add(out=g3, in0=g2, scalar1=GC0 / GC1)
                            vt = vt_pool.tile([P, NC], BF16, name=f"vt{eb}", tag=f"vt{eb}")
                            nc.vector.scalar_tensor_tensor(out=vt, in0=g3, scalar=GC1,
                                                           in1=pvr, op0=ALU.mult,
                                                           op1=ALU.mult)
                            vts.append(vt)
                        return vts

                    def issue_out(c, vts):
                        for s in range(n_sub):
                            for e in range(n_ef):
                                po = out_ps.tile([P, ef], F32, name="po", tag="ops")
                                for eb in range(n_dblk):
                                    nc.tensor.matmul(po, lhsT=vts[eb][:, ts(s, P)],
                                                     rhs=a_tiles[eb][:, ts(e, ef)],
                                                     start=(eb == 0),
                                                     stop=(eb == n_dblk - 1))
                                ot = osb_pool.tile([P, ef], BF16, name="ot", tag="osb")
                                nc.vector.tensor_mul(out=ot, in0=po,
                                                     in1=rq_bc[:, ts(e, ef)])
                                nc.sync.dma_start(
                                    out=out_d[c * NC + s * P:c * NC + (s + 1) * P,
                                              ts(e, ef)],
                                    in_=ot)

                    vts_prev = issue_vrvi(0, xts_cache.pop(0))
                    for c in range(n_chunks):
                        if c + 1 < n_chunks:
                            vts_next = issue_vrvi(c + 1, xts_cache.pop(c + 1))
                        if c + 2 < n_chunks:
                            xts_cache[c + 2] = dma_xt(c + 2)
                        issue_out(c, vts_prev)
                        if c + 1 < n_chunks:
                            vts_prev = vts_next
    nc.compile()
    return nc


_PROGRAM_CACHE = {}


def _get_program(n, d):
    key = (n, d)
    if key not in _PROGRAM_CACHE:
        _PROGRAM_CACHE[key] = build_program(n, d)
    return _PROGRAM_CACHE[key]


def _numpy_reference(x, Wvr, bvr, Wvi, bvi, Wk, bk, Wq, bq):
    """Slow fp32 fallback (never expected to run: biases are zeros)."""
    out = np.empty_like(x)
    for b in range(x.shape[0]):
        xb = x[b].astype(np.float64)
        vr = xb @ Wvr.T.astype(np.float64) + bvr
        vi = xb @ Wvi.T.astype(np.float64) + bvi
        v = vr * np.tanh(np.logaddexp(0.0, vi))
        k = xb @ Wk.T.astype(np.float64) + bk
        q = xb @ Wq.T.astype(np.float64) + bq
        kn = k / (np.linalg.norm(k, axis=0, keepdims=True) + 1e-5)
        qn = q / (np.linalg.norm(q, axis=0, keepdims=True) + 1e-5)
        g = kn.T @ qn
        a = 0.625 * g + 0.375 * np.abs(g)
        out[b] = (v @ a).astype(np.float32)
    return out


def kernel(_run_kwargs=None, **inputs):
    run_kwargs = _run_kwargs or {}
    x = np.asarray(inputs["x"], dtype=np.float32)
    Wvr = np.asarray(inputs["Wvr"], dtype=np.float32)
    Wvi = np.asarray(inputs["Wvi"], dtype=np.float32)
    Wk = np.asarray(inputs["Wk"], dtype=np.float32)
    Wq = np.asarray(inputs["Wq"], dtype=np.float32)
    bvr, bvi = np.asarray(inputs["bvr"]), np.asarray(inputs["bvi"])
    bk, bq = np.asarray(inputs["bk"]), np.asarray(inputs["bq"])

    if any(np.any(b != 0) for b in (bvr, bvi, bk, bq)):
        return _numpy_reference(x, Wvr, bvr, Wvi, bvi, Wk, bk, Wq, bq)

    b, n, d = x.shape
    assert b == B and n == N_FULL and d == D_FULL, (b, n, d)

    bf16 = ml_dtypes.bfloat16
    wvr_t = np.ascontiguousarray(Wvr.T).astype(bf16)
    wvi_t = np.ascontiguousarray(Wvi.T).astype(bf16)
    wk_t = np.ascontiguousarray(Wk.T).astype(bf16)
    wq_t = np.ascontiguousarray(Wq.T).astype(bf16)

    in_maps = []
    for i in range(N_CORES):
        in_maps.append({
            "xn": x[i].astype(bf16),
            "xt": np.ascontiguousarray(x[i].T).astype(bf16),
            "wvr": wvr_t, "wvi": wvi_t, "wk": wk_t, "wq": wq_t,
        })

    nc = _get_program(n, d)
    from concourse.bass_utils import run_bass_kernel_spmd
    res = run_bass_kernel_spmd(nc, in_maps, core_ids=list(range(N_CORES)), **run_kwargs)
    out = np.stack([np.asarray(res.results[i]["out"]).astype(np.float32)
                    for i in range(N_CORES)], axis=0)
    if run_kwargs:
        kernel.last_results = res
    return out


# revision 15
# speedup vs baseline: 1.4922x; 1.0400x over previous
"""Trainium2 Bass kernel for nn_GatedFeedForward (gated feed-forward with
feature attention).

Reference computation per batch b (B=8, N=4096, D=1024):
    VR = x @ Wvr.T ; VI = x @ Wvi.T            (biases are zero)
    V  = VR * tanh(softplus(VI))
    K  = x @ Wk.T  ; Q  = x @ Wq.T
    Kn = K / (||K||_col + 1e-5) ; Qn = Q / (||Q||_col + 1e-5)   (norm over N)
    A  = smu(Kn.T @ Qn)     # == leaky-relu slope 0.25 == 0.625x + 0.375|x|
    out = V @ A
Sharding: pure data-parallel over batch — one batch per NeuronCore.

Key algebraic restructure: with S = X^T X (D x D, one N-contraction),
    K^T Q        = WkT^T S WqT          (WkT = Wk.T, [in,out])
    ||K_d||^2    = colsum(WkT * (S WkT))
    ||Q_e||^2    = colsum(WqT * (S WqT))
so the K/Q path costs one N*D^2 matmul (S) plus three D^3 matmuls
(Tk = S WkT, G = Tk^T WqT, Uq = S WqT) instead of two N*D^2 (K, Q) plus
one N*D^2 (K^T Q): 2.4x less PE time on that path. leaky's positive
homogeneity folds rk into A's rows and rq into the output tiles.

The gate tanh(softplus(x)) is evaluated as c0 + c1*tanh(a1 x + b1)
+ c2*tanh(a2 x + b2) (max abs err 3.9e-3): both ops hit the resident
tanh activation table — no table switches, no slow DVE reciprocal.

Schedule per core (all matmuls bf16 with fp32 PSUM):
  Pass 1: S accumulated directly in PSUM across all 8 sequence chunks,
      in two column-half phases of 8 banks each; cast to bf16 at phase end.
  Mid:    Tk (+nk2 via ones-matmuls), Uq (+nq2), G -> A' from PSUM;
      norm scalars overlap the Uq matmuls on ACT/DVE.
  Pass 2: VRVI(c) / out(c) software-pipelined with lag 1 so the PE never
      waits on the gate chain. Output stored bf16, upcast on host.
"""

import numpy as np
import ml_dtypes

import concourse.bass as bass
import concourse.tile as tile
from concourse import bacc, mybir
from concourse.bass import ts

F32 = mybir.dt.float32
BF16 = mybir.dt.bfloat16
AF = mybir.ActivationFunctionType
ALU = mybir.AluOpType

B = 8
N_FULL = 4096
D_FULL = 1024
N_CORES = 8

P = 128  # SBUF partitions
NC = 512  # sequence chunk
EF = 512  # free-dim span per matmul / psum bank

# tanh-sum fit of tanh(softplus(x)), max |err| 3.9e-3 on [-12, 12]
GC0, GC1, GA1, GB1 = 0.50022747, 0.32785149, 0.8261997, -0.02962021
GC2, GA2, GB2 = 0.17216236, 0.57575332, 0.75023909


def build_program(n=N_FULL, d=D_FULL):
    """Build the single-core SPMD Bass program for one [n, d] batch."""
    assert n % NC == 0 and d % P == 0
    n_chunks = n // NC
    n_sub = NC // P  # 128-row subtiles per chunk
    n_dblk = d // P  # feature blocks
    ef = min(EF, d)
    n_ef = d // ef

    nc = bacc.Bacc("TRN2", target_bir_lowering=False, debug=False,
                   num_devices=N_CORES)
    xn = nc.dram_tensor("xn", [n, d], BF16, kind="ExternalInput")
    xt = nc.dram_tensor("xt", [d, n], BF16, kind="ExternalInput")
    wvr = nc.dram_tensor("wvr", [d, d], BF16, kind="ExternalInput")
    wvi = nc.dram_tensor("wvi", [d, d], BF16, kind="ExternalInput")
    wk = nc.dram_tensor("wk", [d, d], BF16, kind="ExternalInput")
    wq = nc.dram_tensor("wq", [d, d], BF16, kind="ExternalInput")
    out_d = nc.dram_tensor("out", [n, d], BF16, kind="ExternalOutput")

    with tile.TileContext(nc) as tc:
        with tc.tile_pool(name="const", bufs=1) as const_pool, \
             tc.tile_pool(name="w", bufs=1) as w_pool, \
             tc.tile_pool(name="sb", bufs=1) as sb_pool, \
             tc.tile_pool(name="tkb", bufs=1) as tkb_pool, \
             tc.tile_pool(name="post", bufs=1) as post_pool, \
             tc.tile_pool(name="apost", bufs=1) as ap_pool:
            ones32 = const_pool.tile([P, 1], F32, name="ones32", tag="ones32")
            nc.vector.memset(ones32, 1.0)
            one11 = const_pool.tile([1, 1], F32, name="one11", tag="one11")
            nc.vector.memset(one11, 1.0)
            ones_row = const_pool.tile([1, P], F32, name="ones_row", tag="onesr")
            nc.vector.memset(ones_row, 1.0)
            gb1 = const_pool.tile([P, 1], F32, name="gb1", tag="gb1")
            nc.vector.memset(gb1, GB1)
            gb2 = const_pool.tile([P, 1], F32, name="gb2", tag="gb2")
            nc.vector.memset(gb2, GB2)

            w_tiles = {}
            for wname, wdram in (("wk", wk), ("wq", wq), ("wvr", wvr), ("wvi", wvi)):
                w_tiles[wname] = [
                    w_pool.tile([P, d], BF16, name=f"{wname}{db}", tag=f"{wname}{db}")
                    for db in range(n_dblk)
                ]

            sb_tiles = [sb_pool.tile([P, d], BF16, name=f"s{ib}", tag=f"s{ib}")
                        for ib in range(n_dblk)]
            tkb_tiles = [tkb_pool.tile([P, d], BF16, name=f"tk{ib}", tag=f"tk{ib}")
                         for ib in range(n_dblk)]
            a_tiles = [ap_pool.tile([P, d], BF16, name=f"a{db}", tag=f"a{db}")
                       for db in range(n_dblk)]

            # ---------------- Pass 1: S = X^T X, PSUM-resident ----------------
            with tc.tile_pool(name="xn1", bufs=3) as xn_pool, \
                 tc.tile_pool(name="s_ps", bufs=1, space="PSUM") as s_ps:

                def dma_xn(e, c):
                    tiles = []
                    for s in range(n_sub):
                        t = xn_pool.tile([P, d], BF16, name=f"xn{s}", tag=f"xn{s}")
                        nc.sync.dma_start(out=t, in_=xn[c * NC + s * P:c * NC + (s + 1) * P, :])
                        tiles.append(t)
                    return tiles

                seq = [(e, c) for e in range(n_ef) for c in range(n_chunks)]
                xn_cache = {seq[0]: dma_xn(*seq[0])}
                wseq = [("wk", wk), ("wq", wq), ("wvr", wvr), ("wvi", wvi)]
                for idx, (e, c) in enumerate(seq):
                    # prefetch two chunk-sets ahead; weight DMAs interleave one
                    # tensor per chunk so they never block the x stream
                    for j in (idx + 1, idx + 2):
                        if j < len(seq) and seq[j] not in xn_cache:
                            xn_cache[seq[j]] = dma_xn(*seq[j])
                    if 1 <= idx <= len(wseq):
                        wname, wdram = wseq[idx - 1]
                        for db in range(n_dblk):
                            nc.sync.dma_start(out=w_tiles[wname][db],
                                              in_=wdram[ts(db, P), :])
                    if c == 0:
                        ps_list = [s_ps.tile([P, ef], F32, name=f"sps{ib}", tag=f"sps{ib}")
                                   for ib in range(n_dblk)]
                    xns = xn_cache.pop((e, c))
                    for s in range(n_sub):
                        for ib in range(n_dblk):
                            nc.tensor.matmul(ps_list[ib], lhsT=xns[s][:, ts(ib, P)],
                                             rhs=xns[s][:, ts(e, ef)],
                                             start=(c == 0 and s == 0),
                                             stop=(c == n_chunks - 1 and s == n_sub - 1))
                    if c == n_chunks - 1:
                        for ib in range(n_dblk):
                            nc.vector.tensor_copy(out=sb_tiles[ib][:, ts(e, ef)],
                                                  in_=ps_list[ib])

            # ---------------- pass-2 SBUF pools (open early for prefetch) ------
            nk = post_pool.tile([1, d], F32, name="nk", tag="nk")
            nq = post_pool.tile([1, d], F32, name="nq", tag="nq")
            rk_col = post_pool.tile([P, n_dblk], F32, name="rk_col", tag="rk_col")
            rq_bc = post_pool.tile([P, d], F32, name="rq_bc", tag="rq_bc")
            rk625 = post_pool.tile([P, n_dblk], F32, name="rk625", tag="rk625")
            rk375 = post_pool.tile([P, n_dblk], F32, name="rk375", tag="rk375")

            with tc.tile_pool(name="xt2", bufs=2) as xt2_pool, \
                 tc.tile_pool(name="vt", bufs=2) as vt_pool, \
                 tc.tile_pool(name="gate", bufs=2) as gate_pool, \
                 tc.tile_pool(name="wtmp", bufs=4) as wtmp_pool, \
                 tc.tile_pool(name="osb", bufs=3) as osb_pool:

                def dma_xt(c):
                    tiles = []
                    for db in range(n_dblk):
                        t = xt2_pool.tile([P, NC], BF16, name=f"x2_{db}", tag=f"x2_{db}")
                        nc.sync.dma_start(out=t, in_=xt[ts(db, P), ts(c, NC)])
                        tiles.append(t)
                    return tiles

                xts_cache = {0: dma_xt(0), 1: dma_xt(1)}

                def issue_vrvi_pool(c, xts, ps_pool):
                    vts = []
                    for eb in range(n_dblk):
                        pvr = ps_pool.tile([P, NC], F32, name="pvr", tag="midps")
                        pvi = ps_pool.tile([P, NC], F32, name="pvi", tag="midps")
                        for db in range(n_dblk):
                            st = dict(start=(db == 0), stop=(db == n_dblk - 1))
                            nc.tensor.matmul(pvr, lhsT=w_tiles["wvr"][db][:, ts(eb, P)],
                                             rhs=xts[db], **st)
                            nc.tensor.matmul(pvi, lhsT=w_tiles["wvi"][db][:, ts(eb, P)],
                                             rhs=xts[db], **st)
                        # gate = c0 + c1*tanh(a1 x + b1) + c2*tanh(a2 x + b2)
                        t1 = gate_pool.tile([P, NC], F32, name="t1", tag="t1")
                        nc.scalar.activation(out=t1, in_=pvi, func=AF.Tanh,
                                             scale=GA1, bias=gb1)
                        t2 = gate_pool.tile([P, NC], F32, name="t2", tag="t2")
                        nc.scalar.activation(out=t2, in_=pvi, func=AF.Tanh,
                                             scale=GA2, bias=gb2)
                        g1 = gate_pool.tile([P, NC], F32, name="g1", tag="g1")
                        nc.vector.tensor_scalar_mul(out=g1, in0=t2, scalar1=GC2 / GC1)
                        g2 = gate_pool.tile([P, NC], F32, name="g2", tag="g2")
                        nc.vector.tensor_add(out=g2, in0=t1, in1=g1)
                        g3 = gate_pool.tile([P, NC], F32, name="g3", tag="g3")
                        nc.vector.tensor_scalar_add(out=g3, in0=g2, scalar1=GC0 / GC1)
                        vt = vt_pool.tile([P, NC], BF16, name=f"vt{eb}", tag=f"vt{eb}")
                        nc.vector.scalar_tensor_tensor(out=vt, in0=g3, scalar=GC1,
                                                       in1=pvr, op0=ALU.mult,
                                                       op1=ALU.mult)
                        vts.append(vt)
                    return vts

                # ---------------- Mid: Tk, Uq, G, norms, A' ----------------
                with tc.tile_pool(name="mid_ps", bufs=4, space="PSUM") as mid_ps, \
                     tc.tile_pool(name="nrm_ps", bufs=1, space="PSUM") as nrm_ps:

                    # Tk = S @ WkT ; nk2 = colsum(WkT * Tk)
                    # software-pipeline the ones-matmuls one group behind so the
                    # PE never waits on the DVE mult
                    pend = []  # (ones_mm_args) delayed by one group

                    def flush_pend():
                        while pend:
                            pn, tmp, st = pend.pop(0)
                            nc.tensor.matmul(pn, lhsT=ones32, rhs=tmp, **st)

                    nk2_ps = {e: nrm_ps.tile([1, ef], F32, name=f"nk2{e}", tag=f"nrm{e}")
                              for e in range(n_ef)}
                    # ib-outer so the first 8 groups touch only the phase-A
                    # halves of S — the phase-B casts drain underneath them
                    for ib in range(n_dblk):
                        for e in range(n_ef):
                            pt = mid_ps.tile([P, ef], F32, name="pt", tag="midps")
                            for jb in range(n_dblk):
                                nc.tensor.matmul(pt, lhsT=sb_tiles[jb][:, ts(ib, P)],
                                                 rhs=w_tiles["wk"][jb][:, ts(e, ef)],
                                                 start=(jb == 0), stop=(jb == n_dblk - 1))
                            nc.vector.tensor_copy(out=tkb_tiles[ib][:, ts(e, ef)], in_=pt)
                            tmp = wtmp_pool.tile([P, ef], F32, name="tmp", tag="wtmp")
                            nc.vector.tensor_mul(out=tmp, in0=pt,
                                                 in1=w_tiles["wk"][ib][:, ts(e, ef)])
                            flush_pend()
                            pend.append((nk2_ps[e], tmp,
                                         dict(start=(ib == 0), stop=(ib == n_dblk - 1))))
                    flush_pend()

                    # rk chain on ACT/DVE (overlaps the Uq matmuls below)
                    for e in range(n_ef):
                        nc.scalar.activation(out=nk[0:1, ts(e, ef)], in_=nk2_ps[e],
                                             func=AF.Sqrt)
                    nc.vector.tensor_scalar_add(out=nk, in0=nk, scalar1=1e-5)
                    nc.vector.reciprocal(out=nk, in_=nk)

                    # Uq = S @ WqT ; nq2 = colsum(WqT * Uq)
                    nq2_ps = {}
                    for e in range(n_ef):
                        # reuses nk2's bank; allocated after the sqrt reads above
                        nq2_ps[e] = nrm_ps.tile([1, ef], F32, name=f"nq2{e}", tag=f"nrm{e}")
                        for ib in range(n_dblk):
                            pt = mid_ps.tile([P, ef], F32, name="pu", tag="midps")
                            for jb in range(n_dblk):
                                nc.tensor.matmul(pt, lhsT=sb_tiles[jb][:, ts(ib, P)],
                                                 rhs=w_tiles["wq"][jb][:, ts(e, ef)],
                                                 start=(jb == 0), stop=(jb == n_dblk - 1))
                            tmp = wtmp_pool.tile([P, ef], F32, name="tmq", tag="wtmp")
                            nc.vector.tensor_mul(out=tmp, in0=pt,
                                                 in1=w_tiles["wq"][ib][:, ts(e, ef)])
                            flush_pend()
                            pend.append((nq2_ps[e], tmp,
                                         dict(start=(ib == 0), stop=(ib == n_dblk - 1))))
                        if e == 0:
                            # rk transpose to per-partition column (tiny PE matmuls
                            # interleaved between Uq groups)
                            for db in range(n_dblk):
                                pm = mid_ps.tile([P, ef], F32, name="pm", tag="midps")
                                nc.tensor.matmul(pm[:, 0:1], lhsT=nk[0:1, ts(db, P)],
                                                 rhs=one11, start=True, stop=True)
                                nc.vector.tensor_copy(out=rk_col[:, db:db + 1],
                                                      in_=pm[:, 0:1])
                            nc.vector.tensor_scalar_mul(out=rk625, in0=rk_col,
                                                        scalar1=0.625)
                            nc.vector.tensor_scalar_mul(out=rk375, in0=rk_col,
                                                        scalar1=0.375)
                    # G = Tk^T @ WqT ; A' = rk * (0.625 G + 0.375 |G|) from PSUM.
                    # The trailing nq2 ones-matmul, the rq chain and the first
                    # VRVI chunk all interleave with the G groups so the PE
                    # never idles across the mid->pass2 transition.
                    g_groups = [(eb, e) for eb in range(n_dblk) for e in range(n_ef)]
                    for gi, (eb, e) in enumerate(g_groups):
                        pg = mid_ps.tile([P, ef], F32, name="pg", tag="midps")
                        for db in range(n_dblk):
                            nc.tensor.matmul(pg, lhsT=tkb_tiles[db][:, ts(eb, P)],
                                             rhs=w_tiles["wq"][db][:, ts(e, ef)],
                                             start=(db == 0), stop=(db == n_dblk - 1))
                        if gi == 0:
                            flush_pend()
                        if gi == 1:
                            # rq scalar chain on ACT/DVE under the G matmuls
                            for ee in range(n_ef):
                                nc.scalar.activation(out=nq[0:1, ts(ee, ef)],
                                                     in_=nq2_ps[ee], func=AF.Sqrt)
                            nc.vector.tensor_scalar_add(out=nq, in0=nq, scalar1=1e-5)
                            nc.vector.reciprocal(out=nq, in_=nq)
                        if gi == 3:
                            for ee in range(n_ef):
                                pb = mid_ps.tile([P, ef], F32, name="pb", tag="midps")
                                nc.tensor.matmul(pb, lhsT=ones_row,
                                                 rhs=nq[0:1, ts(ee, ef)],
                                                 start=True, stop=True)
                                nc.vector.tensor_copy(out=rq_bc[:, ts(ee, ef)], in_=pb)
                        tabs = wtmp_pool.tile([P, ef], F32, name="tabs", tag="wtmp")
                        nc.scalar.activation(out=tabs, in_=pg, func=AF.Abs,
                                             scale=rk375[:, eb:eb + 1])
                        nc.vector.scalar_tensor_tensor(
                            out=a_tiles[eb][:, ts(e, ef)], in0=pg,
                            scalar=rk625[:, eb:eb + 1], in1=tabs,
                            op0=ALU.mult, op1=ALU.add)

                    # first VRVI chunk straight out of the mid psum pool (same
                    # tile shape) — no pool-boundary bubble
                    vts_first = issue_vrvi_pool(0, xts_cache.pop(0), mid_ps)

                # ---------------- Pass 2: VRVI / out pipeline ----------------
                with tc.tile_pool(name="vrvi_ps", bufs=4, space="PSUM") as vrvi_ps, \
                     tc.tile_pool(name="out_ps", bufs=3, space="PSUM") as out_ps:

                    def issue_vrvi(c, xts):
                        return issue_vrvi_pool(c, xts, vrvi_ps)

                    def issue_out(c, vts):
                        for s in range(n_sub):
                            for e in range(n_ef):
                                po = out_ps.tile([P, ef], F32, name="po", tag="ops")
                                for eb in range(n_dblk):
                                    nc.tensor.matmul(po, lhsT=vts[eb][:, ts(s, P)],
                                                     rhs=a_tiles[eb][:, ts(e, ef)],
                                                     start=(eb == 0),
                                                     stop=(eb == n_dblk - 1))
                                ot = osb_pool.tile([P, ef], BF16, name="ot", tag="osb")
                                nc.vector.tensor_mul(out=ot, in0=po,
                                                     in1=rq_bc[:, ts(e, ef)])
                                nc.sync.dma_start(
                                    out=out_d[c * NC + s * P:c * NC + (s + 1) * P,
                                              ts(e, ef)],
                                    in_=ot)

                    vts_prev = vts_first
                    for c in range(n_chunks):
                        if c + 1 < n_chunks:
                            vts_next = issue_vrvi(c + 1, xts_cache.pop(c + 1))
                        if c + 2 < n_chunks:
                            xts_cache[c + 2] = dma_xt(c + 2)
                        issue_out(c, vts_prev)
                        if c + 1 < n_chunks:
                            vts_prev = vts_next
    nc.compile()
    return nc


_PROGRAM_CACHE = {}


def _get_program(n, d):
    key = (n, d)
    if key not in _PROGRAM_CACHE:
        _PROGRAM_CACHE[key] = build_program(n, d)
    return _PROGRAM_CACHE[key]


def _numpy_reference(x, Wvr, bvr, Wvi, bvi, Wk, bk, Wq, bq):
    """Slow fp32 fallback (never expected to run: biases are zeros)."""
    out = np.empty_like(x)
    for b in range(x.shape[0]):
        xb = x[b].astype(np.float64)
        vr = xb @ Wvr.T.astype(np.float64) + bvr
        vi = xb @ Wvi.T.astype(np.float64) + bvi
        v = vr * np.tanh(np.logaddexp(0.0, vi))
        k = xb @ Wk.T.astype(np.float64) + bk
        q = xb @ Wq.T.astype(np.float64) + bq
        kn = k / (np.linalg.norm(k, axis=0, keepdims=True) + 1e-5)
        qn = q / (np.linalg.norm(q, axis=0, keepdims=True) + 1e-5)
        g = kn.T @ qn
        a = 0.625 * g + 0.375 * np.abs(g)
        out[b] = (v @ a).astype(np.float32)
    return out


def kernel(_run_kwargs=None, **inputs):
    run_kwargs = _run_kwargs or {}
    x = np.asarray(inputs["x"], dtype=np.float32)
    Wvr = np.asarray(inputs["Wvr"], dtype=np.float32)
    Wvi = np.asarray(inputs["Wvi"], dtype=np.float32)
    Wk = np.asarray(inputs["Wk"], dtype=np.float32)
    Wq = np.asarray(inputs["Wq"], dtype=np.float32)
    bvr, bvi = np.asarray(inputs["bvr"]), np.asarray(inputs["bvi"])
    bk, bq = np.asarray(inputs["bk"]), np.asarray(inputs["bq"])

    if any(np.any(b != 0) for b in (bvr, bvi, bk, bq)):
        return _numpy_reference(x, Wvr, bvr, Wvi, bvi, Wk, bk, Wq, bq)

    b, n, d = x.shape
    assert b == B and n == N_FULL and d == D_FULL, (b, n, d)

    bf16 = ml_dtypes.bfloat16
    wvr_t = np.ascontiguousarray(Wvr.T).astype(bf16)
    wvi_t = np.ascontiguousarray(Wvi.T).astype(bf16)
    wk_t = np.ascontiguousarray(Wk.T).astype(bf16)
    wq_t = np.ascontiguousarray(Wq.T).astype(bf16)

    in_maps = []
    for i in range(N_CORES):
        in_maps.append({
            "xn": x[i].astype(bf16),
            "xt": np.ascontiguousarray(x[i].T).astype(bf16),
            "wvr": wvr_t, "wvi": wvi_t, "wk": wk_t, "wq": wq_t,
        })

    nc = _get_program(n, d)
    from concourse.bass_utils import run_bass_kernel_spmd
    res = run_bass_kernel_spmd(nc, in_maps, core_ids=list(range(N_CORES)), **run_kwargs)
    out = np.stack([np.asarray(res.results[i]["out"]).astype(np.float32)
                    for i in range(N_CORES)], axis=0)
    if run_kwargs:
        kernel.last_results = res
    return out


# revision 38
# speedup vs baseline: 1.6716x; 1.1202x over previous
"""Trainium2 Bass kernel for nn_GatedFeedForward (gated feed-forward with
feature attention).

Reference computation per batch b (B=8, N=4096, D=1024):
    VR = x @ Wvr.T ; VI = x @ Wvi.T            (biases are zero)
    V  = VR * tanh(softplus(VI))
    K  = x @ Wk.T  ; Q  = x @ Wq.T
    Kn = K / (||K||_col + 1e-5) ; Qn = Q / (||Q||_col + 1e-5)   (norm over N)
    A  = smu(Kn.T @ Qn)     # == leaky-relu slope 0.25 == 0.625x + 0.375|x|
    out = V @ A
Sharding: pure data-parallel over batch — one batch per NeuronCore.

Key algebraic restructure: with S = X^T X (D x D, one N-contraction),
    K^T Q        = WkT^T S WqT          (WkT = Wk.T, [in,out])
    ||K_d||^2    = colsum(WkT * (S WkT))
    ||Q_e||^2    = colsum(WqT * (S WqT))
so the K/Q path costs one N*D^2 matmul (S) plus three D^3 matmuls
(Tk = S WkT, G = Tk^T WqT, Uq = S WqT) instead of two N*D^2 (K, Q) plus
one N*D^2 (K^T Q): 2.4x less PE time on that path. leaky's positive
homogeneity folds rk into A's rows and rq into the output tiles.

The gate tanh(softplus(x)) is evaluated as c0 + c1*tanh(a1 x + b1)
+ c2*tanh(a2 x + b2) (max abs err 3.9e-3): both ops hit the resident
tanh activation table — no table switches, no slow DVE reciprocal.

Schedule per core (all matmuls bf16 with fp32 PSUM):
  Pass 1: S accumulated directly in PSUM across all 8 sequence chunks,
      in two column-half phases of 8 banks each; cast to bf16 at phase end.
  Mid:    Tk (+nk2 via ones-matmuls), Uq (+nq2), G -> A' from PSUM;
      norm scalars overlap the Uq matmuls on ACT/DVE.
  Pass 2: VRVI(c) / out(c) software-pipelined with lag 1 so the PE never
      waits on the gate chain. Output stored bf16, upcast on host.
"""

import numpy as np
import ml_dtypes

import concourse.bass as bass
import concourse.tile as tile
from concourse import bacc, mybir
from concourse.bass import ts

F32 = mybir.dt.float32
BF16 = mybir.dt.bfloat16
AF = mybir.ActivationFunctionType
ALU = mybir.AluOpType

B = 8
N_FULL = 4096
D_FULL = 1024
N_CORES = 8

P = 128  # SBUF partitions
NC = 512  # sequence chunk
EF = 512  # free-dim span per matmul / psum bank

# tanh-sum fit of tanh(softplus(x)), max |err| 3.9e-3 on [-12, 12]
GC0, GC1, GA1, GB1 = 0.50022747, 0.32785149, 0.8261997, -0.02962021
GC2, GA2, GB2 = 0.17216236, 0.57575332, 0.75023909


def build_program(n=N_FULL, d=D_FULL):
    """Build the single-core SPMD Bass program for one [n, d] batch."""
    assert n % NC == 0 and d % P == 0
    n_chunks = n // NC
    n_sub = NC // P  # 128-row subtiles per chunk
    n_dblk = d // P  # feature blocks
    ef = min(EF, d)
    n_ef = d // ef

    nc = bacc.Bacc("TRN2", target_bir_lowering=False, debug=False,
                   num_devices=N_CORES)
    ident = nc.dram_tensor("ident", [P, P], BF16, kind="ExternalInput")
    xn = nc.dram_tensor("xn", [n, d], BF16, kind="ExternalInput")
    xt = nc.dram_tensor("xt", [d, n], BF16, kind="ExternalInput")
    wvr = nc.dram_tensor("wvr", [d, d], BF16, kind="ExternalInput")
    wvi = nc.dram_tensor("wvi", [d, d], BF16, kind="ExternalInput")
    wk = nc.dram_tensor("wk", [d, d], BF16, kind="ExternalInput")
    wq = nc.dram_tensor("wq", [d, d], BF16, kind="ExternalInput")
    out_d = nc.dram_tensor("out", [n, d], BF16, kind="ExternalOutput")

    with tile.TileContext(nc) as tc:
        with tc.tile_pool(name="const", bufs=1) as const_pool, \
             tc.tile_pool(name="w", bufs=1) as w_pool, \
             tc.tile_pool(name="sb", bufs=1) as sb_pool, \
             tc.tile_pool(name="tkb", bufs=1) as tkb_pool, \
             tc.tile_pool(name="post", bufs=1) as post_pool, \
             tc.tile_pool(name="apost", bufs=1) as ap_pool:
            ones32 = const_pool.tile([P, 1], F32, name="ones32", tag="ones32")
            nc.vector.memset(ones32, 1.0)
            # bf16 ones for the norm partition-reduce matmuls: fp32 moving
            # operands stream at half rate and add PE dtype-mode switches
            onesb = const_pool.tile([P, 1], BF16, name="onesb", tag="onesb")
            nc.vector.memset(onesb, 1.0)
            one11 = const_pool.tile([1, 1], F32, name="one11", tag="one11")
            nc.vector.memset(one11, 1.0)
            ones_row = const_pool.tile([1, P], F32, name="ones_row", tag="onesr")
            nc.vector.memset(ones_row, 1.0)
            gb1 = const_pool.tile([P, 1], F32, name="gb1", tag="gb1")
            nc.vector.memset(gb1, GB1)
            gb2 = const_pool.tile([P, 1], F32, name="gb2", tag="gb2")
            nc.vector.memset(gb2, GB2)
            ident_sb = const_pool.tile([P, P], BF16, name="ident", tag="ident")

            w_tiles = {}
            for wname, wdram in (("wk", wk), ("wq", wq), ("wvr", wvr), ("wvi", wvi)):
                w_tiles[wname] = [
                    w_pool.tile([P, d], BF16, name=f"{wname}{db}", tag=f"{wname}{db}")
                    for db in range(n_dblk)
                ]

            sb_tiles = [sb_pool.tile([P, d], BF16, name=f"s{ib}", tag=f"s{ib}")
                        for ib in range(n_dblk)]
            tkb_tiles = [tkb_pool.tile([P, d], BF16, name=f"tk{ib}", tag=f"tk{ib}")
                         for ib in range(n_dblk)]
            a_tiles = [ap_pool.tile([P, d], BF16, name=f"a{db}", tag=f"a{db}")
                       for db in range(n_dblk)]

            # ---------------- Pass 1: S = X^T X, PSUM-resident ----------------
            with tc.tile_pool(name="xn1", bufs=3) as xn_pool, \
                 tc.tile_pool(name="s_ps", bufs=1, space="PSUM") as s_ps:

                def dma_xn(e, c):
                    tiles = []
                    for s in range(n_sub):
                        t = xn_pool.tile([P, d], BF16, name=f"xn{s}", tag=f"xn{s}")
                        nc.sync.dma_start(out=t, in_=xn[c * NC + s * P:c * NC + (s + 1) * P, :])
                        tiles.append(t)
                    return tiles

                # S is symmetric: compute only blocks with 128*ib <= 512*e+511
                # (the right half for all rows, then the upper-left quadrant)
                # and reconstruct the lower-left quadrant by PE transposes.
                # Right-half phase (8 psum banks) runs first so the transposes
                # and the 4-bank upper-left phase overlap cleanly.
                ib_set = {e: list(range(min(n_dblk, (ef * (e + 1)) // P)))
                          for e in range(n_ef)}
                seq = [(e, c) for e in sorted(range(n_ef), reverse=True)
                       for c in range(n_chunks)]
                xn_cache = {seq[0]: dma_xn(*seq[0])}
                # wk/wq stream during pass 1; wvr/wvi are issued at mid start
                wseq = [("wk", wk), ("wq", wq)]
                for idx, (e, c) in enumerate(seq):
                    # prefetch two chunk-sets ahead; weight DMAs interleave half
                    # a tensor per chunk so they never block the x stream
                    for j in (idx + 1, idx + 2):
                        if j < len(seq) and seq[j] not in xn_cache:
                            xn_cache[seq[j]] = dma_xn(*seq[j])
                    if idx == 1:
                        nc.sync.dma_start(out=ident_sb, in_=ident[:, :])
                    if 2 <= idx <= 2 * len(wseq) + 1:
                        wname, wdram = wseq[(idx - 2) // 2]
                        half = (idx - 2) % 2
                        for db in range(half * n_dblk // 2, (half + 1) * n_dblk // 2):
                            nc.sync.dma_start(out=w_tiles[wname][db],
                                              in_=wdram[ts(db, P), :])
                    if c == 0:
                        ps_list = {ib: s_ps.tile([P, ef], F32, name=f"sps{ib}",
                                                 tag=f"sps{ib}")
                                   for ib in ib_set[e]}
                    xns = xn_cache.pop((e, c))
                    last = c == n_chunks - 1
                    # last chunk runs ib-outer so each accumulator group stops
                    # early and its cast drains under the remaining matmuls
                    order = ([(s, ib) for ib in ib_set[e] for s in range(n_sub)]
                             if last else
                             [(s, ib) for s in range(n_sub) for ib in ib_set[e]])
                    for s, ib in order:
                        nc.tensor.matmul(ps_list[ib], lhsT=xns[s][:, ts(ib, P)],
                                         rhs=xns[s][:, ts(e, ef)],
                                         start=(c == 0 and s == 0),
                                         stop=(last and s == n_sub - 1))
                        if last and s == n_sub - 1:
                            # alternate engines: halves the serial cast chain
                            if ib % 2 == 0:
                                nc.vector.tensor_copy(out=sb_tiles[ib][:, ts(e, ef)],
                                                      in_=ps_list[ib])
                            else:
                                nc.scalar.activation(out=sb_tiles[ib][:, ts(e, ef)],
                                                     in_=ps_list[ib], func=AF.Copy)

            # ---------------- pass-2 SBUF pools (open early for prefetch) ------
            nk = post_pool.tile([1, d], F32, name="nk", tag="nk")
            nq = post_pool.tile([1, d], F32, name="nq", tag="nq")
            rk_col = post_pool.tile([P, n_dblk], F32, name="rk_col", tag="rk_col")
            rq_bc = post_pool.tile([P, d], F32, name="rq_bc", tag="rq_bc")
            rk625 = post_pool.tile([P, n_dblk], F32, name="rk625", tag="rk625")
            rk375 = post_pool.tile([P, n_dblk], F32, name="rk375", tag="rk375")

            with tc.tile_pool(name="xt2", bufs=2) as xt2_pool, \
                 tc.tile_pool(name="vt", bufs=2) as vt_pool, \
                 tc.tile_pool(name="gate", bufs=2) as gate_pool, \
                 tc.tile_pool(name="wtmp", bufs=4) as wtmp_pool, \
                 tc.tile_pool(name="osb", bufs=3) as osb_pool:

                def dma_xt(c):
                    tiles = []
                    for db in range(n_dblk):
                        t = xt2_pool.tile([P, NC], BF16, name=f"x2_{db}", tag=f"x2_{db}")
                        nc.sync.dma_start(out=t, in_=xt[ts(db, P), ts(c, NC)])
                        tiles.append(t)
                    return tiles

                xts_cache = {0: dma_xt(0), 1: dma_xt(1)}

                def issue_vrvi_pool(c, xts, ps_pool):
                    vts = []
                    for eb in range(n_dblk):
                        pvr = ps_pool.tile([P, NC], F32, name="pvr", tag="midps")
                        pvi = ps_pool.tile([P, NC], F32, name="pvi", tag="midps")
                        for db in range(n_dblk):
                            st = dict(start=(db == 0), stop=(db == n_dblk - 1))
                            nc.tensor.matmul(pvr, lhsT=w_tiles["wvr"][db][:, ts(eb, P)],
                                             rhs=xts[db], **st)
                            nc.tensor.matmul(pvi, lhsT=w_tiles["wvi"][db][:, ts(eb, P)],
                                             rhs=xts[db], **st)
                        # gate = c0 + c1*tanh(a1 x + b1) + c2*tanh(a2 x + b2)
                        t1 = gate_pool.tile([P, NC], F32, name="t1", tag="t1")
                        nc.scalar.activation(out=t1, in_=pvi, func=AF.Tanh,
                                             scale=GA1, bias=gb1)
                        t2 = gate_pool.tile([P, NC], F32, name="t2", tag="t2")
                        nc.scalar.activation(out=t2, in_=pvi, func=AF.Tanh,
                                             scale=GA2, bias=gb2)
                        g2 = gate_pool.tile([P, NC], F32, name="g2", tag="g2")
                        nc.vector.scalar_tensor_tensor(out=g2, in0=t2, scalar=GC2 / GC1,
                                                       in1=t1, op0=ALU.mult, op1=ALU.add)
                        g3 = gate_pool.tile([P, NC], F32, name="g3", tag="g3")
                        nc.vector.tensor_scalar(out=g3, in0=g2, scalar1=GC1,
                                                scalar2=GC0, op0=ALU.mult, op1=ALU.add)
                        vt = vt_pool.tile([P, NC], BF16, name=f"vt{eb}", tag=f"vt{eb}")
                        nc.vector.tensor_mul(out=vt, in0=g3, in1=pvr)
                        vts.append(vt)
                    return vts

                # ---------------- Mid: Tk, Uq, G, norms, A' ----------------
                with tc.tile_pool(name="mid_ps", bufs=5, space="PSUM") as mid_ps, \
                     tc.tile_pool(name="tp_ps", bufs=1, space="PSUM") as tp_ps, \
                     tc.tile_pool(name="nrm_ps", bufs=1, space="PSUM") as nrm_ps:

                    # wvr/wvi stream in under the Tk matmuls; first needed by
                    # VRVI(0) at the end of the mid phase
                    for wname, wdram in (("wvr", wvr), ("wvi", wvi)):
                        for db in range(n_dblk):
                            nc.sync.dma_start(out=w_tiles[wname][db],
                                              in_=wdram[ts(db, P), :])

                    # lower-left S blocks = transposes of the computed
                    # upper-right ones (S is symmetric, values bit-identical)
                    pairs = [(n_dblk // 2 + i, jb) for jb in range(n_dblk // 2)
                             for i in range(n_dblk // 2)]
                    for g in range(0, len(pairs), 4):
                        ptp = tp_ps.tile([P, ef], BF16, name="tps", tag="tps")
                        for k, (ibp, jb) in enumerate(pairs[g:g + 4]):
                            nc.tensor.transpose(out=ptp[:, ts(k, P)],
                                                in_=sb_tiles[jb][:, ts(ibp, P)],
                                                identity=ident_sb)
                        for k, (ibp, jb) in enumerate(pairs[g:g + 4]):
                            nc.vector.tensor_copy(out=sb_tiles[ibp][:, ts(jb, P)],
                                                  in_=ptp[:, ts(k, P)])

                    # Tk = S @ WkT ; nk2 = colsum(WkT * Tk)
                    # software-pipeline the ones-matmuls one group behind so the
                    # PE never waits on the DVE mult
                    pend = []  # (ones_mm_args) delayed by one group

                    def flush_pend():
                        while pend:
                            pn, tmp, st = pend.pop(0)
                            nc.tensor.matmul(pn, lhsT=onesb, rhs=tmp, **st)

                    nk2_ps = {e: nrm_ps.tile([1, ef], F32, name=f"nk2{e}", tag=f"nrm{e}")
                              for e in range(n_ef)}
                    # ib-outer so the first 8 groups touch only the phase-A
                    # halves of S — the phase-B casts drain underneath them
                    for ib in range(n_dblk):
                        for e in range(n_ef):
                            pt = mid_ps.tile([P, ef], F32, name="pt", tag="midps")
                            for jb in range(n_dblk):
                                nc.tensor.matmul(pt, lhsT=sb_tiles[jb][:, ts(ib, P)],
                                                 rhs=w_tiles["wk"][jb][:, ts(e, ef)],
                                                 start=(jb == 0), stop=(jb == n_dblk - 1))
                            nc.vector.tensor_copy(out=tkb_tiles[ib][:, ts(e, ef)], in_=pt)
                            tmp = wtmp_pool.tile([P, ef], BF16, name="tmp", tag="wtmpb")
                            nc.vector.tensor_mul(out=tmp, in0=pt,
                                                 in1=w_tiles["wk"][ib][:, ts(e, ef)])
                            flush_pend()
                            pend.append((nk2_ps[e], tmp,
                                         dict(start=(ib == 0), stop=(ib == n_dblk - 1))))
                    flush_pend()

                    # rk = 1/sqrt(nk2) in one ACT table op (the +1e-5 in the
                    # reference is a 1.6e-7 relative effect at these norms).
                    # A DVE reciprocal on [1, d] runs on a single partition
                    # (~6.5us!) and head-of-line-blocks the DVE queue.
                    for ee in range(n_ef):
                        nc.scalar.activation(out=nk[0:1, ts(ee, ef)], in_=nk2_ps[ee],
                                             func=AF.Abs_reciprocal_sqrt)

                    # Uq = S @ WqT ; nq2 = colsum(WqT * Uq)
                    nq2_ps = {}
                    for e in range(n_ef):
                        # reuses nk2's bank; allocated after the sqrt reads above
                        nq2_ps[e] = nrm_ps.tile([1, ef], F32, name=f"nq2{e}", tag=f"nrm{e}")
                        for ib in range(n_dblk):
                            pt = mid_ps.tile([P, ef], F32, name="pu", tag="midps")
                            for jb in range(n_dblk):
                                nc.tensor.matmul(pt, lhsT=sb_tiles[jb][:, ts(ib, P)],
                                                 rhs=w_tiles["wq"][jb][:, ts(e, ef)],
                                                 start=(jb == 0), stop=(jb == n_dblk - 1))
                            tmp = wtmp_pool.tile([P, ef], BF16, name="tmq", tag="wtmpb")
                            nc.vector.tensor_mul(out=tmp, in0=pt,
                                                 in1=w_tiles["wq"][ib][:, ts(e, ef)])
                            flush_pend()
                            pend.append((nq2_ps[e], tmp,
                                         dict(start=(ib == 0), stop=(ib == n_dblk - 1))))
                        if e == 0:
                            # rk transpose to per-partition column (tiny PE matmuls
                            # interleaved between Uq groups)
                            for db in range(n_dblk):
                                pm = mid_ps.tile([P, ef], F32, name="pm", tag="midps")
                                nc.tensor.matmul(pm[:, 0:1], lhsT=nk[0:1, ts(db, P)],
                                                 rhs=one11, start=True, stop=True)
                                nc.vector.tensor_copy(out=rk_col[:, db:db + 1],
                                                      in_=pm[:, 0:1])
                            nc.vector.tensor_scalar_mul(out=rk625, in0=rk_col,
                                                        scalar1=0.625)
                            nc.vector.tensor_scalar_mul(out=rk375, in0=rk_col,
                                                        scalar1=0.375)
                    # G = Tk^T @ WqT ; A' = rk * (0.625 G + 0.375 |G|) from PSUM.
                    # The trailing nq2 ones-matmul, the rq chain and the first
                    # VRVI chunk all interleave with the G groups so the PE
                    # never idles across the mid->pass2 transition.
                    g_groups = [(eb, e) for eb in range(n_dblk) for e in range(n_ef)]
                    for gi, (eb, e) in enumerate(g_groups):
                        pg = mid_ps.tile([P, ef], F32, name="pg", tag="midps")
                        for db in range(n_dblk):
                            nc.tensor.matmul(pg, lhsT=tkb_tiles[db][:, ts(eb, P)],
                                             rhs=w_tiles["wq"][db][:, ts(e, ef)],
                                             start=(db == 0), stop=(db == n_dblk - 1))
                        if gi == 0:
                            flush_pend()
                        if gi == 1:
                            # rq = 1/sqrt(nq2) on ACT under the G matmuls
                            for ee in range(n_ef):
                                nc.scalar.activation(out=nq[0:1, ts(ee, ef)],
                                                     in_=nq2_ps[ee],
                                                     func=AF.Abs_reciprocal_sqrt)
                        if gi == 3:
                            for ee in range(n_ef):
                                pb = mid_ps.tile([P, ef], F32, name="pb", tag="midps")
                                nc.tensor.matmul(pb, lhsT=ones_row,
                                                 rhs=nq[0:1, ts(ee, ef)],
                                                 start=True, stop=True)
                                nc.vector.tensor_copy(out=rq_bc[:, ts(ee, ef)], in_=pb)
                        tabs = wtmp_pool.tile([P, ef], F32, name="tabs", tag="wtmp")
                        nc.scalar.activation(out=tabs, in_=pg, func=AF.Abs,
                                             scale=rk375[:, eb:eb + 1])
                        nc.vector.scalar_tensor_tensor(
                            out=a_tiles[eb][:, ts(e, ef)], in0=pg,
                            scalar=rk625[:, eb:eb + 1], in1=tabs,
                            op0=ALU.mult, op1=ALU.add)

                    # first two VRVI chunks straight out of the mid psum pool
                    # (same tile shape) — in the pass-2 loop each further VRVI
                    # chunk then follows a full out() phase, so its psum ring
                    # reuse never waits on a trailing gate chain
                    vts_fifo = [issue_vrvi_pool(0, xts_cache.pop(0), mid_ps),
                                issue_vrvi_pool(1, xts_cache.pop(1), mid_ps)]

                # ---------------- Pass 2: VRVI / out pipeline ----------------
                with tc.tile_pool(name="vrvi_ps", bufs=4, space="PSUM") as vrvi_ps, \
                     tc.tile_pool(name="out_ps", bufs=4, space="PSUM") as out_ps:

                    def issue_vrvi(c, xts):
                        return issue_vrvi_pool(c, xts, vrvi_ps)

                    def issue_out(c, vts):
                        for s in range(n_sub):
                            for e in range(n_ef):
                                po = out_ps.tile([P, ef], F32, name="po", tag="ops")
                                for eb in range(n_dblk):
                                    nc.tensor.matmul(po, lhsT=vts[eb][:, ts(s, P)],
                                                     rhs=a_tiles[eb][:, ts(e, ef)],
                                                     start=(eb == 0),
                                                     stop=(eb == n_dblk - 1))
                                ot = osb_pool.tile([P, ef], BF16, name="ot", tag="osb")
                                nc.vector.tensor_mul(out=ot, in0=po,
                                                     in1=rq_bc[:, ts(e, ef)])
                                nc.sync.dma_start(
                                    out=out_d[c * NC + s * P:c * NC + (s + 1) * P,
                                              ts(e, ef)],
                                    in_=ot)

                    for c in range(n_chunks):
                        if c + 2 < n_chunks:
                            xts_cache[c + 2] = dma_xt(c + 2)
                        issue_out(c, vts_fifo[c])
                        if c + 2 < n_chunks:
                            vts_fifo.append(issue_vrvi(c + 2, xts_cache.pop(c + 2)))
    nc.compile()
    return nc


_PROGRAM_CACHE = {}


def _get_program(n, d):
    key = (n, d)
    if key not in _PROGRAM_CACHE:
        _PROGRAM_CACHE[key] = build_program(n, d)
    return _PROGRAM_CACHE[key]


def _numpy_reference(x, Wvr, bvr, Wvi, bvi, Wk, bk, Wq, bq):
    """Slow fp32 fallback (never expected to run: biases are zeros)."""
    out = np.empty_like(x)
    for b in range(x.shape[0]):
        xb = x[b].astype(np.float64)
        vr = xb @ Wvr.T.astype(np.float64) + bvr
        vi = xb @ Wvi.T.astype(np.float64) + bvi
        v = vr * np.tanh(np.logaddexp(0.0, vi))
        k = xb @ Wk.T.astype(np.float64) + bk
        q = xb @ Wq.T.astype(np.float64) + bq
        kn = k / (np.linalg.norm(k, axis=0, keepdims=True) + 1e-5)
        qn = q / (np.linalg.norm(q, axis=0, keepdims=True) + 1e-5)
        g = kn.T @ qn
        a = 0.625 * g + 0.375 * np.abs(g)
        out[b] = (v @ a).astype(np.float32)
    return out


def kernel(_run_kwargs=None, **inputs):
    run_kwargs = _run_kwargs or {}
    x = np.asarray(inputs["x"], dtype=np.float32)
    Wvr = np.asarray(inputs["Wvr"], dtype=np.float32)
    Wvi = np.asarray(inputs["Wvi"], dtype=np.float32)
    Wk = np.asarray(inputs["Wk"], dtype=np.float32)
    Wq = np.asarray(inputs["Wq"], dtype=np.float32)
    bvr, bvi = np.asarray(inputs["bvr"]), np.asarray(inputs["bvi"])
    bk, bq = np.asarray(inputs["bk"]), np.asarray(inputs["bq"])

    if any(np.any(b != 0) for b in (bvr, bvi, bk, bq)):
        return _numpy_reference(x, Wvr, bvr, Wvi, bvi, Wk, bk, Wq, bq)

    b, n, d = x.shape
    assert b == B and n == N_FULL and d == D_FULL, (b, n, d)

    bf16 = ml_dtypes.bfloat16
    wvr_t = np.ascontiguousarray(Wvr.T).astype(bf16)
    wvi_t = np.ascontiguousarray(Wvi.T).astype(bf16)
    wk_t = np.ascontiguousarray(Wk.T).astype(bf16)
    wq_t = np.ascontiguousarray(Wq.T).astype(bf16)

    ident = np.eye(P, dtype=bf16)
    in_maps = []
    for i in range(N_CORES):
        in_maps.append({
            "xn": x[i].astype(bf16),
            "xt": np.ascontiguousarray(x[i].T).astype(bf16),
            "wvr": wvr_t, "wvi": wvi_t, "wk": wk_t, "wq": wq_t,
            "ident": ident,
        })

    nc = _get_program(n, d)
    from concourse.bass_utils import run_bass_kernel_spmd
    res = run_bass_kernel_spmd(nc, in_maps, core_ids=list(range(N_CORES)), **run_kwargs)
    out = np.stack([np.asarray(res.results[i]["out"]).astype(np.float32)
                    for i in range(N_CORES)], axis=0)
    if run_kwargs:
        kernel.last_results = res
    return out


# revision 40
# speedup vs baseline: 1.6879x; 1.0097x over previous
"""Trainium2 Bass kernel for nn_GatedFeedForward (gated feed-forward with
feature attention).

Reference computation per batch b (B=8, N=4096, D=1024):
    VR = x @ Wvr.T ; VI = x @ Wvi.T            (biases are zero)
    V  = VR * tanh(softplus(VI))
    K  = x @ Wk.T  ; Q  = x @ Wq.T
    Kn = K / (||K||_col + 1e-5) ; Qn = Q / (||Q||_col + 1e-5)   (norm over N)
    A  = smu(Kn.T @ Qn)     # == leaky-relu slope 0.25 == 0.625x + 0.375|x|
    out = V @ A
Sharding: pure data-parallel over batch — one batch per NeuronCore.

Key algebraic restructure: with S = X^T X (D x D, one N-contraction),
    K^T Q        = WkT^T S WqT          (WkT = Wk.T, [in,out])
    ||K_d||^2    = colsum(WkT * (S WkT))
    ||Q_e||^2    = colsum(WqT * (S WqT))
so the K/Q path costs one N*D^2 matmul (S) plus three D^3 matmuls
(Tk = S WkT, G = Tk^T WqT, Uq = S WqT) instead of two N*D^2 (K, Q) plus
one N*D^2 (K^T Q): 2.4x less PE time on that path. leaky's positive
homogeneity folds rk into A's rows and rq into the output tiles.

The gate tanh(softplus(x)) is evaluated as c0 + c1*tanh(a1 x + b1)
+ c2*tanh(a2 x + b2) (max abs err 3.9e-3): both ops hit the resident
tanh activation table — no table switches, no slow DVE reciprocal.

Schedule per core (all matmuls bf16 with fp32 PSUM):
  Pass 1: S accumulated directly in PSUM across all 8 sequence chunks,
      in two column-half phases of 8 banks each; cast to bf16 at phase end.
  Mid:    Tk (+nk2 via ones-matmuls), Uq (+nq2), G -> A' from PSUM;
      norm scalars overlap the Uq matmuls on ACT/DVE.
  Pass 2: VRVI(c) / out(c) software-pipelined with lag 1 so the PE never
      waits on the gate chain. Output stored bf16, upcast on host.
"""

import numpy as np
import ml_dtypes

import concourse.bass as bass
import concourse.tile as tile
from concourse import bacc, mybir
from concourse.bass import ts

F32 = mybir.dt.float32
BF16 = mybir.dt.bfloat16
AF = mybir.ActivationFunctionType
ALU = mybir.AluOpType

B = 8
N_FULL = 4096
D_FULL = 1024
N_CORES = 8

P = 128  # SBUF partitions
NC = 512  # sequence chunk
EF = 512  # free-dim span per matmul / psum bank

# tanh-sum fit of tanh(softplus(x)), max |err| 3.9e-3 on [-12, 12]
GC0, GC1, GA1, GB1 = 0.50022747, 0.32785149, 0.8261997, -0.02962021
GC2, GA2, GB2 = 0.17216236, 0.57575332, 0.75023909


def build_program(n=N_FULL, d=D_FULL):
    """Build the single-core SPMD Bass program for one [n, d] batch."""
    assert n % NC == 0 and d % P == 0
    n_chunks = n // NC
    n_sub = NC // P  # 128-row subtiles per chunk
    n_dblk = d // P  # feature blocks
    ef = min(EF, d)
    n_ef = d // ef

    nc = bacc.Bacc("TRN2", target_bir_lowering=False, debug=False,
                   num_devices=N_CORES)
    ident = nc.dram_tensor("ident", [P, P], BF16, kind="ExternalInput")
    xn = nc.dram_tensor("xn", [n, d], BF16, kind="ExternalInput")
    xt = nc.dram_tensor("xt", [d, n], BF16, kind="ExternalInput")
    wvr = nc.dram_tensor("wvr", [d, d], BF16, kind="ExternalInput")
    wvi = nc.dram_tensor("wvi", [d, d], BF16, kind="ExternalInput")
    wk = nc.dram_tensor("wk", [d, d], BF16, kind="ExternalInput")
    wq = nc.dram_tensor("wq", [d, d], BF16, kind="ExternalInput")
    out_d = nc.dram_tensor("out", [n, d], BF16, kind="ExternalOutput")

    with tile.TileContext(nc) as tc:
        with tc.tile_pool(name="const", bufs=1) as const_pool, \
             tc.tile_pool(name="w", bufs=1) as w_pool, \
             tc.tile_pool(name="sb", bufs=1) as sb_pool, \
             tc.tile_pool(name="tkb", bufs=1) as tkb_pool, \
             tc.tile_pool(name="post", bufs=1) as post_pool, \
             tc.tile_pool(name="apost", bufs=1) as ap_pool:
            ones32 = const_pool.tile([P, 1], F32, name="ones32", tag="ones32")
            nc.vector.memset(ones32, 1.0)
            # bf16 ones for the norm partition-reduce matmuls: fp32 moving
            # operands stream at half rate and add PE dtype-mode switches
            onesb = const_pool.tile([P, 1], BF16, name="onesb", tag="onesb")
            nc.vector.memset(onesb, 1.0)
            one11 = const_pool.tile([1, 1], F32, name="one11", tag="one11")
            nc.vector.memset(one11, 1.0)
            ones_row = const_pool.tile([1, P], F32, name="ones_row", tag="onesr")
            nc.vector.memset(ones_row, 1.0)
            gb1 = const_pool.tile([P, 1], F32, name="gb1", tag="gb1")
            nc.vector.memset(gb1, GB1)
            gb2 = const_pool.tile([P, 1], F32, name="gb2", tag="gb2")
            nc.vector.memset(gb2, GB2)
            ident_sb = const_pool.tile([P, P], BF16, name="ident", tag="ident")

            w_tiles = {}
            for wname, wdram in (("wk", wk), ("wq", wq), ("wvr", wvr), ("wvi", wvi)):
                w_tiles[wname] = [
                    w_pool.tile([P, d], BF16, name=f"{wname}{db}", tag=f"{wname}{db}")
                    for db in range(n_dblk)
                ]

            sb_tiles = [sb_pool.tile([P, d], BF16, name=f"s{ib}", tag=f"s{ib}")
                        for ib in range(n_dblk)]
            tkb_tiles = [tkb_pool.tile([P, d], BF16, name=f"tk{ib}", tag=f"tk{ib}")
                         for ib in range(n_dblk)]
            a_tiles = [ap_pool.tile([P, d], BF16, name=f"a{db}", tag=f"a{db}")
                       for db in range(n_dblk)]

            # ---------------- Pass 1: S = X^T X, PSUM-resident ----------------
            with tc.tile_pool(name="xn1", bufs=3) as xn_pool, \
                 tc.tile_pool(name="s_ps", bufs=1, space="PSUM") as s_ps:

                def dma_xn(e, c):
                    tiles = []
                    for s in range(n_sub):
                        t = xn_pool.tile([P, d], BF16, name=f"xn{s}", tag=f"xn{s}")
                        nc.sync.dma_start(out=t, in_=xn[c * NC + s * P:c * NC + (s + 1) * P, :])
                        tiles.append(t)
                    return tiles

                # S is symmetric: compute only blocks with 128*ib <= 512*e+511
                # (the right half for all rows, then the upper-left quadrant)
                # and reconstruct the lower-left quadrant by PE transposes.
                # Right-half phase (8 psum banks) runs first so the transposes
                # and the 4-bank upper-left phase overlap cleanly.
                ib_set = {e: list(range(min(n_dblk, (ef * (e + 1)) // P)))
                          for e in range(n_ef)}
                seq = [(e, c) for e in sorted(range(n_ef), reverse=True)
                       for c in range(n_chunks)]
                xn_cache = {seq[0]: dma_xn(*seq[0])}
                # wk/wq stream during pass 1; wvr/wvi are issued at mid start
                wseq = [("wk", wk), ("wq", wq)]
                for idx, (e, c) in enumerate(seq):
                    # prefetch two chunk-sets ahead; weight DMAs interleave half
                    # a tensor per chunk so they never block the x stream
                    for j in (idx + 1, idx + 2):
                        if j < len(seq) and seq[j] not in xn_cache:
                            xn_cache[seq[j]] = dma_xn(*seq[j])
                    if idx == 1:
                        nc.sync.dma_start(out=ident_sb, in_=ident[:, :])
                    if 2 <= idx <= 2 * len(wseq) + 1:
                        wname, wdram = wseq[(idx - 2) // 2]
                        half = (idx - 2) % 2
                        for db in range(half * n_dblk // 2, (half + 1) * n_dblk // 2):
                            nc.sync.dma_start(out=w_tiles[wname][db],
                                              in_=wdram[ts(db, P), :])
                    if c == 0:
                        ps_list = {ib: s_ps.tile([P, ef], F32, name=f"sps{ib}",
                                                 tag=f"sps{ib}")
                                   for ib in ib_set[e]}
                    xns = xn_cache.pop((e, c))
                    last = c == n_chunks - 1
                    # last chunk runs ib-outer so each accumulator group stops
                    # early and its cast drains under the remaining matmuls
                    order = ([(s, ib) for ib in ib_set[e] for s in range(n_sub)]
                             if last else
                             [(s, ib) for s in range(n_sub) for ib in ib_set[e]])
                    for s, ib in order:
                        nc.tensor.matmul(ps_list[ib], lhsT=xns[s][:, ts(ib, P)],
                                         rhs=xns[s][:, ts(e, ef)],
                                         start=(c == 0 and s == 0),
                                         stop=(last and s == n_sub - 1))
                        if last and s == n_sub - 1:
                            # alternate engines: halves the serial cast chain
                            if ib % 2 == 0:
                                nc.vector.tensor_copy(out=sb_tiles[ib][:, ts(e, ef)],
                                                      in_=ps_list[ib])
                            else:
                                nc.scalar.activation(out=sb_tiles[ib][:, ts(e, ef)],
                                                     in_=ps_list[ib], func=AF.Copy)

            # ---------------- pass-2 SBUF pools (open early for prefetch) ------
            nk = post_pool.tile([1, d], F32, name="nk", tag="nk")
            nq = post_pool.tile([1, d], F32, name="nq", tag="nq")
            rk_col = post_pool.tile([P, n_dblk], F32, name="rk_col", tag="rk_col")
            rq_bc = post_pool.tile([P, d], F32, name="rq_bc", tag="rq_bc")
            rk625 = post_pool.tile([P, n_dblk], F32, name="rk625", tag="rk625")
            rk375 = post_pool.tile([P, n_dblk], F32, name="rk375", tag="rk375")

            with tc.tile_pool(name="xt2", bufs=2) as xt2_pool, \
                 tc.tile_pool(name="vt", bufs=2) as vt_pool, \
                 tc.tile_pool(name="gate", bufs=2) as gate_pool, \
                 tc.tile_pool(name="wtmp", bufs=4) as wtmp_pool, \
                 tc.tile_pool(name="osb", bufs=3) as osb_pool:

                def dma_xt(c):
                    tiles = []
                    for db in range(n_dblk):
                        t = xt2_pool.tile([P, NC], BF16, name=f"x2_{db}", tag=f"x2_{db}")
                        nc.sync.dma_start(out=t, in_=xt[ts(db, P), ts(c, NC)])
                        tiles.append(t)
                    return tiles

                xts_cache = {0: dma_xt(0), 1: dma_xt(1)}

                def issue_vrvi_pool(c, xts, ps_pool):
                    vts = []
                    for eb in range(n_dblk):
                        pvr = ps_pool.tile([P, NC], F32, name="pvr", tag="midps")
                        pvi = ps_pool.tile([P, NC], F32, name="pvi", tag="midps")
                        for db in range(n_dblk):
                            st = dict(start=(db == 0), stop=(db == n_dblk - 1))
                            nc.tensor.matmul(pvr, lhsT=w_tiles["wvr"][db][:, ts(eb, P)],
                                             rhs=xts[db], **st)
                            nc.tensor.matmul(pvi, lhsT=w_tiles["wvi"][db][:, ts(eb, P)],
                                             rhs=xts[db], **st)
                        # gate = c0 + c1*tanh(a1 x + b1) + c2*tanh(a2 x + b2)
                        t1 = gate_pool.tile([P, NC], F32, name="t1", tag="t1")
                        nc.scalar.activation(out=t1, in_=pvi, func=AF.Tanh,
                                             scale=GA1, bias=gb1)
                        t2 = gate_pool.tile([P, NC], F32, name="t2", tag="t2")
                        nc.scalar.activation(out=t2, in_=pvi, func=AF.Tanh,
                                             scale=GA2, bias=gb2)
                        g2 = gate_pool.tile([P, NC], F32, name="g2", tag="g2")
                        nc.vector.scalar_tensor_tensor(out=g2, in0=t2, scalar=GC2 / GC1,
                                                       in1=t1, op0=ALU.mult, op1=ALU.add)
                        g3 = gate_pool.tile([P, NC], F32, name="g3", tag="g3")
                        nc.vector.tensor_scalar(out=g3, in0=g2, scalar1=GC1,
                                                scalar2=GC0, op0=ALU.mult, op1=ALU.add)
                        vt = vt_pool.tile([P, NC], BF16, name=f"vt{eb}", tag=f"vt{eb}")
                        nc.vector.tensor_mul(out=vt, in0=g3, in1=pvr)
                        vts.append(vt)
                    return vts

                # ---------------- Mid: Tk, Uq, G, norms, A' ----------------
                with tc.tile_pool(name="mid_ps", bufs=5, space="PSUM") as mid_ps, \
                     tc.tile_pool(name="tp_ps", bufs=1, space="PSUM") as tp_ps, \
                     tc.tile_pool(name="nrm_ps", bufs=1, space="PSUM") as nrm_ps:

                    # wvr/wvi stream in under the Tk matmuls; first needed by
                    # VRVI(0) at the end of the mid phase
                    for wname, wdram in (("wvr", wvr), ("wvi", wvi)):
                        for db in range(n_dblk):
                            nc.sync.dma_start(out=w_tiles[wname][db],
                                              in_=wdram[ts(db, P), :])

                    # lower-left S blocks = transposes of the computed
                    # upper-right ones (S is symmetric, values bit-identical)
                    pairs = [(n_dblk // 2 + i, jb) for jb in range(n_dblk // 2)
                             for i in range(n_dblk // 2)]
                    for g in range(0, len(pairs), 4):
                        ptp = tp_ps.tile([P, ef], BF16, name="tps", tag="tps")
                        for k, (ibp, jb) in enumerate(pairs[g:g + 4]):
                            nc.tensor.transpose(out=ptp[:, ts(k, P)],
                                                in_=sb_tiles[jb][:, ts(ibp, P)],
                                                identity=ident_sb)
                        for k, (ibp, jb) in enumerate(pairs[g:g + 4]):
                            nc.vector.tensor_copy(out=sb_tiles[ibp][:, ts(jb, P)],
                                                  in_=ptp[:, ts(k, P)])

                    # Tk = S @ WkT ; nk2 = colsum(WkT * Tk)
                    # software-pipeline the ones-matmuls one group behind so the
                    # PE never waits on the DVE mult
                    pend = []  # (ones_mm_args) delayed by one group

                    def flush_pend():
                        while pend:
                            pn, tmp, st = pend.pop(0)
                            nc.tensor.matmul(pn, lhsT=onesb, rhs=tmp, **st)

                    nk2_ps = {e: nrm_ps.tile([1, ef], F32, name=f"nk2{e}", tag=f"nrm{e}")
                              for e in range(n_ef)}
                    # ib-outer so the first 8 groups touch only the phase-A
                    # halves of S — the phase-B casts drain underneath them
                    for ib in range(n_dblk):
                        for e in range(n_ef):
                            pt = mid_ps.tile([P, ef], F32, name="pt", tag="midps")
                            for jb in range(n_dblk):
                                nc.tensor.matmul(pt, lhsT=sb_tiles[jb][:, ts(ib, P)],
                                                 rhs=w_tiles["wk"][jb][:, ts(e, ef)],
                                                 start=(jb == 0), stop=(jb == n_dblk - 1))
                            nc.vector.tensor_copy(out=tkb_tiles[ib][:, ts(e, ef)], in_=pt)
                            tmp = wtmp_pool.tile([P, ef], BF16, name="tmp", tag="wtmpb")
                            nc.vector.tensor_mul(out=tmp, in0=pt,
                                                 in1=w_tiles["wk"][ib][:, ts(e, ef)])
                            flush_pend()
                            pend.append((nk2_ps[e], tmp,
                                         dict(start=(ib == 0), stop=(ib == n_dblk - 1))))
                    flush_pend()

                    # rk = 1/sqrt(nk2) in one ACT table op (the +1e-5 in the
                    # reference is a 1.6e-7 relative effect at these norms).
                    # A DVE reciprocal on [1, d] runs on a single partition
                    # (~6.5us!) and head-of-line-blocks the DVE queue.
                    for ee in range(n_ef):
                        nc.scalar.activation(out=nk[0:1, ts(ee, ef)], in_=nk2_ps[ee],
                                             func=AF.Abs_reciprocal_sqrt)

                    # Uq = S @ WqT ; nq2 = colsum(WqT * Uq)
                    nq2_ps = {}
                    for e in range(n_ef):
                        # reuses nk2's bank; allocated after the sqrt reads above
                        nq2_ps[e] = nrm_ps.tile([1, ef], F32, name=f"nq2{e}", tag=f"nrm{e}")
                        for ib in range(n_dblk):
                            pt = mid_ps.tile([P, ef], F32, name="pu", tag="midps")
                            for jb in range(n_dblk):
                                nc.tensor.matmul(pt, lhsT=sb_tiles[jb][:, ts(ib, P)],
                                                 rhs=w_tiles["wq"][jb][:, ts(e, ef)],
                                                 start=(jb == 0), stop=(jb == n_dblk - 1))
                            tmp = wtmp_pool.tile([P, ef], BF16, name="tmq", tag="wtmpb")
                            nc.vector.tensor_mul(out=tmp, in0=pt,
                                                 in1=w_tiles["wq"][ib][:, ts(e, ef)])
                            flush_pend()
                            pend.append((nq2_ps[e], tmp,
                                         dict(start=(ib == 0), stop=(ib == n_dblk - 1))))
                        if e == 0:
                            # rk transpose to per-partition column (tiny PE matmuls
                            # interleaved between Uq groups)
                            for db in range(n_dblk):
                                pm = mid_ps.tile([P, ef], F32, name="pm", tag="midps")
                                nc.tensor.matmul(pm[:, 0:1], lhsT=nk[0:1, ts(db, P)],
                                                 rhs=one11, start=True, stop=True)
                                nc.vector.tensor_copy(out=rk_col[:, db:db + 1],
                                                      in_=pm[:, 0:1])
                            nc.vector.tensor_scalar_mul(out=rk625, in0=rk_col,
                                                        scalar1=0.625)
                            nc.vector.tensor_scalar_mul(out=rk375, in0=rk_col,
                                                        scalar1=0.375)
                    # G = Tk^T @ WqT ; A' = rk * (0.625 G + 0.375 |G|) from PSUM.
                    # The trailing nq2 ones-matmul, the rq chain and the first
                    # VRVI chunk all interleave with the G groups so the PE
                    # never idles across the mid->pass2 transition.
                    g_groups = [(eb, e) for eb in range(n_dblk) for e in range(n_ef)]
                    for gi, (eb, e) in enumerate(g_groups):
                        pg = mid_ps.tile([P, ef], F32, name="pg", tag="midps")
                        for db in range(n_dblk):
                            nc.tensor.matmul(pg, lhsT=tkb_tiles[db][:, ts(eb, P)],
                                             rhs=w_tiles["wq"][db][:, ts(e, ef)],
                                             start=(db == 0), stop=(db == n_dblk - 1))
                        if gi == 0:
                            flush_pend()
                        if gi == 1:
                            # rq = 1/sqrt(nq2) on ACT under the G matmuls
                            for ee in range(n_ef):
                                nc.scalar.activation(out=nq[0:1, ts(ee, ef)],
                                                     in_=nq2_ps[ee],
                                                     func=AF.Abs_reciprocal_sqrt)
                        if gi == 3:
                            for ee in range(n_ef):
                                pb = mid_ps.tile([P, ef], F32, name="pb", tag="midps")
                                nc.tensor.matmul(pb, lhsT=ones_row,
                                                 rhs=nq[0:1, ts(ee, ef)],
                                                 start=True, stop=True)
                                nc.vector.tensor_copy(out=rq_bc[:, ts(ee, ef)], in_=pb)
                        tabs = wtmp_pool.tile([P, ef], F32, name="tabs", tag="wtmp")
                        nc.scalar.activation(out=tabs, in_=pg, func=AF.Abs,
                                             scale=rk375[:, eb:eb + 1])
                        nc.vector.scalar_tensor_tensor(
                            out=a_tiles[eb][:, ts(e, ef)], in0=pg,
                            scalar=rk625[:, eb:eb + 1], in1=tabs,
                            op0=ALU.mult, op1=ALU.add)

                    def issue_out(c, vts, ps_pool):
                        for s in range(n_sub):
                            for e in range(n_ef):
                                po = ps_pool.tile([P, ef], F32, name="po", tag="midps")
                                for eb in range(n_dblk):
                                    nc.tensor.matmul(po, lhsT=vts[eb][:, ts(s, P)],
                                                     rhs=a_tiles[eb][:, ts(e, ef)],
                                                     start=(eb == 0),
                                                     stop=(eb == n_dblk - 1))
                                ot = osb_pool.tile([P, ef], BF16, name="ot", tag="osb")
                                nc.vector.tensor_mul(out=ot, in0=po,
                                                     in1=rq_bc[:, ts(e, ef)])
                                nc.sync.dma_start(
                                    out=out_d[c * NC + s * P:c * NC + (s + 1) * P,
                                              ts(e, ef)],
                                    in_=ot)

                    # first two VRVI chunks AND the first out phase run out of
                    # the mid psum pool (same tile shape): every later pool-
                    # boundary bank reuse is then separated from its previous
                    # consumer by a full VRVI or out phase of PE work, so the
                    # transition never waits on a trailing gate chain
                    vts_fifo = [issue_vrvi_pool(0, xts_cache.pop(0), mid_ps),
                                issue_vrvi_pool(1, xts_cache.pop(1), mid_ps)]
                    xts_cache[2] = dma_xt(2)
                    issue_out(0, vts_fifo[0], mid_ps)

                # ---------------- Pass 2: VRVI / out pipeline ----------------
                with tc.tile_pool(name="vrvi_ps", bufs=4, space="PSUM") as vrvi_ps, \
                     tc.tile_pool(name="out_ps", bufs=4, space="PSUM") as out_ps:

                    def issue_vrvi(c, xts):
                        return issue_vrvi_pool(c, xts, vrvi_ps)

                    for c in range(1, n_chunks):
                        if c + 1 < n_chunks:
                            vts_fifo.append(issue_vrvi(c + 1, xts_cache.pop(c + 1)))
                        if c + 2 < n_chunks:
                            xts_cache[c + 2] = dma_xt(c + 2)
                        issue_out(c, vts_fifo[c], out_ps)
    nc.compile()
    return nc


_PROGRAM_CACHE = {}


def _get_program(n, d):
    key = (n, d)
    if key not in _PROGRAM_CACHE:
        _PROGRAM_CACHE[key] = build_program(n, d)
    return _PROGRAM_CACHE[key]


def _numpy_reference(x, Wvr, bvr, Wvi, bvi, Wk, bk, Wq, bq):
    """Slow fp32 fallback (never expected to run: biases are zeros)."""
    out = np.empty_like(x)
    for b in range(x.shape[0]):
        xb = x[b].astype(np.float64)
        vr = xb @ Wvr.T.astype(np.float64) + bvr
        vi = xb @ Wvi.T.astype(np.float64) + bvi
        v = vr * np.tanh(np.logaddexp(0.0, vi))
        k = xb @ Wk.T.astype(np.float64) + bk
        q = xb @ Wq.T.astype(np.float64) + bq
        kn = k / (np.linalg.norm(k, axis=0, keepdims=True) + 1e-5)
        qn = q / (np.linalg.norm(q, axis=0, keepdims=True) + 1e-5)
        g = kn.T @ qn
        a = 0.625 * g + 0.375 * np.abs(g)
        out[b] = (v @ a).astype(np.float32)
    return out


def kernel(_run_kwargs=None, **inputs):
    run_kwargs = _run_kwargs or {}
    x = np.asarray(inputs["x"], dtype=np.float32)
    Wvr = np.asarray(inputs["Wvr"], dtype=np.float32)
    Wvi = np.asarray(inputs["Wvi"], dtype=np.float32)
    Wk = np.asarray(inputs["Wk"], dtype=np.float32)
    Wq = np.asarray(inputs["Wq"], dtype=np.float32)
    bvr, bvi = np.asarray(inputs["bvr"]), np.asarray(inputs["bvi"])
    bk, bq = np.asarray(inputs["bk"]), np.asarray(inputs["bq"])

    if any(np.any(b != 0) for b in (bvr, bvi, bk, bq)):
        return _numpy_reference(x, Wvr, bvr, Wvi, bvi, Wk, bk, Wq, bq)

    b, n, d = x.shape
    assert b == B and n == N_FULL and d == D_FULL, (b, n, d)

    bf16 = ml_dtypes.bfloat16
    wvr_t = np.ascontiguousarray(Wvr.T).astype(bf16)
    wvi_t = np.ascontiguousarray(Wvi.T).astype(bf16)
    wk_t = np.ascontiguousarray(Wk.T).astype(bf16)
    wq_t = np.ascontiguousarray(Wq.T).astype(bf16)

    ident = np.eye(P, dtype=bf16)
    in_maps = []
    for i in range(N_CORES):
        in_maps.append({
            "xn": x[i].astype(bf16),
            "xt": np.ascontiguousarray(x[i].T).astype(bf16),
            "wvr": wvr_t, "wvi": wvi_t, "wk": wk_t, "wq": wq_t,
            "ident": ident,
        })

    nc = _get_program(n, d)
    from concourse.bass_utils import run_bass_kernel_spmd
    res = run_bass_kernel_spmd(nc, in_maps, core_ids=list(range(N_CORES)), **run_kwargs)
    out = np.stack([np.asarray(res.results[i]["out"]).astype(np.float32)
                    for i in range(N_CORES)], axis=0)
    if run_kwargs:
        kernel.last_results = res
    return out


# revision 42
# speedup vs baseline: 1.7325x; 1.0265x over previous
"""Trainium2 Bass kernel for nn_GatedFeedForward (gated feed-forward with
feature attention).

Reference computation per batch b (B=8, N=4096, D=1024):
    VR = x @ Wvr.T ; VI = x @ Wvi.T            (biases are zero)
    V  = VR * tanh(softplus(VI))
    K  = x @ Wk.T  ; Q  = x @ Wq.T
    Kn = K / (||K||_col + 1e-5) ; Qn = Q / (||Q||_col + 1e-5)   (norm over N)
    A  = smu(Kn.T @ Qn)     # == leaky-relu slope 0.25 == 0.625x + 0.375|x|
    out = V @ A
Sharding: pure data-parallel over batch — one batch per NeuronCore.

Key algebraic restructure: with S = X^T X (D x D, one N-contraction),
    K^T Q        = WkT^T S WqT          (WkT = Wk.T, [in,out])
    ||K_d||^2    = colsum(WkT * (S WkT))
    ||Q_e||^2    = colsum(WqT * (S WqT))
so the K/Q path costs one N*D^2 matmul (S) plus three D^3 matmuls
(Tk = S WkT, G = Tk^T WqT, Uq = S WqT) instead of two N*D^2 (K, Q) plus
one N*D^2 (K^T Q): 2.4x less PE time on that path. leaky's positive
homogeneity folds rk into A's rows and rq into the output tiles.

The gate tanh(softplus(x)) is evaluated as c0 + c1*tanh(a1 x + b1)
+ c2*tanh(a2 x + b2) (max abs err 3.9e-3): both ops hit the resident
tanh activation table — no table switches, no slow DVE reciprocal.

Schedule per core (all matmuls bf16 with fp32 PSUM):
  Pass 1: S accumulated directly in PSUM across all 8 sequence chunks,
      in two column-half phases of 8 banks each; cast to bf16 at phase end.
  Mid:    Tk (+nk2 via ones-matmuls), Uq (+nq2), G -> A' from PSUM;
      norm scalars overlap the Uq matmuls on ACT/DVE.
  Pass 2: VRVI(c) / out(c) software-pipelined with lag 1 so the PE never
      waits on the gate chain. Output stored bf16, upcast on host.
"""

import numpy as np
import ml_dtypes

import concourse.bass as bass
import concourse.tile as tile
from concourse import bacc, mybir
from concourse.bass import ts

F32 = mybir.dt.float32
BF16 = mybir.dt.bfloat16
AF = mybir.ActivationFunctionType
ALU = mybir.AluOpType

B = 8
N_FULL = 4096
D_FULL = 1024
N_CORES = 8

P = 128  # SBUF partitions
NC = 512  # sequence chunk
EF = 512  # free-dim span per matmul / psum bank

# tanh-sum fit of tanh(softplus(x)), max |err| 3.9e-3 on [-12, 12]
GC0, GC1, GA1, GB1 = 0.50022747, 0.32785149, 0.8261997, -0.02962021
GC2, GA2, GB2 = 0.17216236, 0.57575332, 0.75023909


def build_program(n=N_FULL, d=D_FULL):
    """Build the single-core SPMD Bass program for one [n, d] batch."""
    assert n % NC == 0 and d % P == 0
    n_chunks = n // NC
    n_sub = NC // P  # 128-row subtiles per chunk
    n_dblk = d // P  # feature blocks
    ef = min(EF, d)
    n_ef = d // ef

    nc = bacc.Bacc("TRN2", target_bir_lowering=False, debug=False,
                   num_devices=N_CORES)
    ident = nc.dram_tensor("ident", [P, P], BF16, kind="ExternalInput")
    xn = nc.dram_tensor("xn", [n, d], BF16, kind="ExternalInput")
    xt = nc.dram_tensor("xt", [d, n], BF16, kind="ExternalInput")
    wvr = nc.dram_tensor("wvr", [d, d], BF16, kind="ExternalInput")
    wvi = nc.dram_tensor("wvi", [d, d], BF16, kind="ExternalInput")
    wk = nc.dram_tensor("wk", [d, d], BF16, kind="ExternalInput")
    wq = nc.dram_tensor("wq", [d, d], BF16, kind="ExternalInput")
    out_d = nc.dram_tensor("out", [n, d], BF16, kind="ExternalOutput")

    with tile.TileContext(nc) as tc:
        with tc.tile_pool(name="const", bufs=1) as const_pool, \
             tc.tile_pool(name="w", bufs=1) as w_pool, \
             tc.tile_pool(name="sb", bufs=1) as sb_pool, \
             tc.tile_pool(name="tkb", bufs=1) as tkb_pool, \
             tc.tile_pool(name="post", bufs=1) as post_pool, \
             tc.tile_pool(name="apost", bufs=1) as ap_pool:
            ones32 = const_pool.tile([P, 1], F32, name="ones32", tag="ones32")
            nc.vector.memset(ones32, 1.0)
            # bf16 ones for the norm partition-reduce matmuls: fp32 moving
            # operands stream at half rate and add PE dtype-mode switches
            onesb = const_pool.tile([P, 1], BF16, name="onesb", tag="onesb")
            nc.vector.memset(onesb, 1.0)
            one11 = const_pool.tile([1, 1], F32, name="one11", tag="one11")
            nc.vector.memset(one11, 1.0)
            ones_row = const_pool.tile([1, P], F32, name="ones_row", tag="onesr")
            nc.vector.memset(ones_row, 1.0)
            gb1 = const_pool.tile([P, 1], F32, name="gb1", tag="gb1")
            nc.vector.memset(gb1, GB1)
            gb2 = const_pool.tile([P, 1], F32, name="gb2", tag="gb2")
            nc.vector.memset(gb2, GB2)
            ident_sb = const_pool.tile([P, P], BF16, name="ident", tag="ident")

            w_tiles = {}
            for wname, wdram in (("wk", wk), ("wq", wq), ("wvr", wvr), ("wvi", wvi)):
                w_tiles[wname] = [
                    w_pool.tile([P, d], BF16, name=f"{wname}{db}", tag=f"{wname}{db}")
                    for db in range(n_dblk)
                ]

            sb_tiles = [sb_pool.tile([P, d], BF16, name=f"s{ib}", tag=f"s{ib}")
                        for ib in range(n_dblk)]
            tkb_tiles = [tkb_pool.tile([P, d], BF16, name=f"tk{ib}", tag=f"tk{ib}")
                         for ib in range(n_dblk)]
            a_tiles = [ap_pool.tile([P, d], BF16, name=f"a{db}", tag=f"a{db}")
                       for db in range(n_dblk)]

            # ---------------- Pass 1: S = X^T X, PSUM-resident ----------------
            with tc.tile_pool(name="xn1", bufs=3) as xn_pool, \
                 tc.tile_pool(name="s_ps", bufs=1, space="PSUM") as s_ps:

                def dma_xn(e, c):
                    tiles = []
                    for s in range(n_sub):
                        t = xn_pool.tile([P, d], BF16, name=f"xn{s}", tag=f"xn{s}")
                        nc.sync.dma_start(out=t, in_=xn[c * NC + s * P:c * NC + (s + 1) * P, :])
                        tiles.append(t)
                    return tiles

                # S is symmetric: compute only blocks with 128*ib <= 512*e+511
                # (the right half for all rows, then the upper-left quadrant)
                # and reconstruct the lower-left quadrant by PE transposes.
                # Right-half phase (8 psum banks) runs first so the transposes
                # and the 4-bank upper-left phase overlap cleanly.
                ib_set = {e: list(range(min(n_dblk, (ef * (e + 1)) // P)))
                          for e in range(n_ef)}
                seq = [(e, c) for e in sorted(range(n_ef), reverse=True)
                       for c in range(n_chunks)]
                xn_cache = {seq[0]: dma_xn(*seq[0])}
                # wk/wq stream during pass 1; wvr/wvi are issued at mid start
                wseq = [("wk", wk), ("wq", wq)]
                for idx, (e, c) in enumerate(seq):
                    # prefetch two chunk-sets ahead; weight DMAs interleave half
                    # a tensor per chunk so they never block the x stream
                    for j in (idx + 1, idx + 2):
                        if j < len(seq) and seq[j] not in xn_cache:
                            xn_cache[seq[j]] = dma_xn(*seq[j])
                    if idx == 1:
                        nc.sync.dma_start(out=ident_sb, in_=ident[:, :])
                    if 2 <= idx <= 2 * len(wseq) + 1:
                        wname, wdram = wseq[(idx - 2) // 2]
                        half = (idx - 2) % 2
                        for db in range(half * n_dblk // 2, (half + 1) * n_dblk // 2):
                            nc.sync.dma_start(out=w_tiles[wname][db],
                                              in_=wdram[ts(db, P), :])
                    if c == 0:
                        ps_list = {ib: s_ps.tile([P, ef], F32, name=f"sps{ib}",
                                                 tag=f"sps{ib}")
                                   for ib in ib_set[e]}
                    xns = xn_cache.pop((e, c))
                    last = c == n_chunks - 1
                    # last chunk runs ib-outer so each accumulator group stops
                    # early and its cast drains under the remaining matmuls
                    order = ([(s, ib) for ib in ib_set[e] for s in range(n_sub)]
                             if last else
                             [(s, ib) for s in range(n_sub) for ib in ib_set[e]])
                    for s, ib in order:
                        nc.tensor.matmul(ps_list[ib], lhsT=xns[s][:, ts(ib, P)],
                                         rhs=xns[s][:, ts(e, ef)],
                                         start=(c == 0 and s == 0),
                                         stop=(last and s == n_sub - 1))
                        if last and s == n_sub - 1:
                            # alternate engines: halves the serial cast chain
                            if ib % 2 == 0:
                                nc.vector.tensor_copy(out=sb_tiles[ib][:, ts(e, ef)],
                                                      in_=ps_list[ib])
                            else:
                                nc.scalar.activation(out=sb_tiles[ib][:, ts(e, ef)],
                                                     in_=ps_list[ib], func=AF.Copy)

            # ---------------- pass-2 SBUF pools (open early for prefetch) ------
            nk = post_pool.tile([1, d], F32, name="nk", tag="nk")
            nq = post_pool.tile([1, d], F32, name="nq", tag="nq")
            rk_col = post_pool.tile([P, n_dblk], F32, name="rk_col", tag="rk_col")
            rq_bc = post_pool.tile([P, d], F32, name="rq_bc", tag="rq_bc")
            rk625 = post_pool.tile([P, n_dblk], F32, name="rk625", tag="rk625")
            rk375 = post_pool.tile([P, n_dblk], F32, name="rk375", tag="rk375")

            with tc.tile_pool(name="xt2", bufs=2) as xt2_pool, \
                 tc.tile_pool(name="vt", bufs=2) as vt_pool, \
                 tc.tile_pool(name="gate", bufs=2) as gate_pool, \
                 tc.tile_pool(name="wtmp", bufs=4) as wtmp_pool, \
                 tc.tile_pool(name="osb", bufs=3) as osb_pool:

                def dma_xt(c):
                    tiles = []
                    for db in range(n_dblk):
                        t = xt2_pool.tile([P, NC], BF16, name=f"x2_{db}", tag=f"x2_{db}")
                        nc.sync.dma_start(out=t, in_=xt[ts(db, P), ts(c, NC)])
                        tiles.append(t)
                    return tiles

                xts_cache = {0: dma_xt(0), 1: dma_xt(1)}

                def issue_vrvi_pool(c, xts, ps_pool):
                    vts = []
                    for eb in range(n_dblk):
                        pvr = ps_pool.tile([P, NC], F32, name="pvr", tag="midps")
                        pvi = ps_pool.tile([P, NC], F32, name="pvi", tag="midps")
                        for db in range(n_dblk):
                            st = dict(start=(db == 0), stop=(db == n_dblk - 1))
                            nc.tensor.matmul(pvr, lhsT=w_tiles["wvr"][db][:, ts(eb, P)],
                                             rhs=xts[db], **st)
                            nc.tensor.matmul(pvi, lhsT=w_tiles["wvi"][db][:, ts(eb, P)],
                                             rhs=xts[db], **st)
                        # gate = c0 + c1*tanh(a1 x + b1) + c2*tanh(a2 x + b2)
                        t1 = gate_pool.tile([P, NC], F32, name="t1", tag="t1")
                        nc.scalar.activation(out=t1, in_=pvi, func=AF.Tanh,
                                             scale=GA1, bias=gb1)
                        t2 = gate_pool.tile([P, NC], F32, name="t2", tag="t2")
                        nc.scalar.activation(out=t2, in_=pvi, func=AF.Tanh,
                                             scale=GA2, bias=gb2)
                        g2 = gate_pool.tile([P, NC], F32, name="g2", tag="g2")
                        nc.vector.scalar_tensor_tensor(out=g2, in0=t2, scalar=GC2 / GC1,
                                                       in1=t1, op0=ALU.mult, op1=ALU.add)
                        g3 = gate_pool.tile([P, NC], F32, name="g3", tag="g3")
                        nc.vector.tensor_scalar(out=g3, in0=g2, scalar1=GC1,
                                                scalar2=GC0, op0=ALU.mult, op1=ALU.add)
                        vt = vt_pool.tile([P, NC], BF16, name=f"vt{eb}", tag=f"vt{eb}")
                        nc.vector.tensor_mul(out=vt, in0=g3, in1=pvr)
                        vts.append(vt)
                    return vts

                # ---------------- Mid: Tk, Uq, G, norms, A' ----------------
                with tc.tile_pool(name="mid_ps", bufs=5, space="PSUM") as mid_ps, \
                     tc.tile_pool(name="tp_ps", bufs=1, space="PSUM") as tp_ps, \
                     tc.tile_pool(name="nrm_ps", bufs=1, space="PSUM") as nrm_ps:

                    # wvr/wvi stream in under the Tk matmuls; first needed by
                    # VRVI(0) at the end of the mid phase
                    for wname, wdram in (("wvr", wvr), ("wvi", wvi)):
                        for db in range(n_dblk):
                            nc.sync.dma_start(out=w_tiles[wname][db],
                                              in_=wdram[ts(db, P), :])

                    # lower-left S blocks = transposes of the computed
                    # upper-right ones (S is symmetric, values bit-identical)
                    pairs = [(n_dblk // 2 + i, jb) for jb in range(n_dblk // 2)
                             for i in range(n_dblk // 2)]
                    for g in range(0, len(pairs), 4):
                        ptp = tp_ps.tile([P, ef], BF16, name="tps", tag="tps")
                        for k, (ibp, jb) in enumerate(pairs[g:g + 4]):
                            nc.tensor.transpose(out=ptp[:, ts(k, P)],
                                                in_=sb_tiles[jb][:, ts(ibp, P)],
                                                identity=ident_sb)
                        for k, (ibp, jb) in enumerate(pairs[g:g + 4]):
                            nc.vector.tensor_copy(out=sb_tiles[ibp][:, ts(jb, P)],
                                                  in_=ptp[:, ts(k, P)])

                    # Tk = S @ WkT ; nk2 = colsum(WkT * Tk).
                    # The W*T products accumulate on the DVE in bf16 (partition
                    # sums commute across ib blocks), so the partition-reduce is
                    # ONE ones-matmul per span instead of one per group — the
                    # per-group ones-matmuls cost ~0.4us each in stream time
                    # plus un-hidden LDWEIGHTS bubbles.
                    acc_k = {e: post_pool.tile([P, ef], BF16, name=f"acck{e}",
                                               tag=f"acck{e}") for e in range(n_ef)}
                    acc_q = {e: post_pool.tile([P, ef], BF16, name=f"accq{e}",
                                               tag=f"accq{e}") for e in range(n_ef)}
                    nk2_ps = {e: nrm_ps.tile([1, ef], F32, name=f"nk2{e}", tag=f"nrm{e}")
                              for e in range(n_ef)}
                    # ib-outer so the first 8 groups touch only the phase-A
                    # halves of S — the phase-B casts drain underneath them
                    for ib in range(n_dblk):
                        for e in range(n_ef):
                            pt = mid_ps.tile([P, ef], F32, name="pt", tag="midps")
                            for jb in range(n_dblk):
                                nc.tensor.matmul(pt, lhsT=sb_tiles[jb][:, ts(ib, P)],
                                                 rhs=w_tiles["wk"][jb][:, ts(e, ef)],
                                                 start=(jb == 0), stop=(jb == n_dblk - 1))
                            # cast on ACT (idle here) to keep the DVE under the
                            # matmul pace with the accumulate added
                            nc.scalar.activation(out=tkb_tiles[ib][:, ts(e, ef)],
                                                 in_=pt, func=AF.Copy)
                            tmp = wtmp_pool.tile([P, ef], BF16, name="tmp", tag="wtmpb")
                            nc.vector.tensor_mul(out=tmp, in0=pt,
                                                 in1=w_tiles["wk"][ib][:, ts(e, ef)])
                            if ib == 0:
                                nc.vector.tensor_copy(out=acc_k[e], in_=tmp)
                            else:
                                nc.vector.tensor_add(out=acc_k[e], in0=acc_k[e], in1=tmp)

                    # Uq = S @ WqT ; nq2 = colsum(WqT * Uq)
                    nq2_ps = {}
                    for e in range(n_ef):
                        for ib in range(n_dblk):
                            pt = mid_ps.tile([P, ef], F32, name="pu", tag="midps")
                            for jb in range(n_dblk):
                                nc.tensor.matmul(pt, lhsT=sb_tiles[jb][:, ts(ib, P)],
                                                 rhs=w_tiles["wq"][jb][:, ts(e, ef)],
                                                 start=(jb == 0), stop=(jb == n_dblk - 1))
                            tmp = wtmp_pool.tile([P, ef], BF16, name="tmq", tag="wtmpb")
                            nc.vector.tensor_mul(out=tmp, in0=pt,
                                                 in1=w_tiles["wq"][ib][:, ts(e, ef)])
                            if ib == 0:
                                nc.vector.tensor_copy(out=acc_q[e], in_=tmp)
                            else:
                                nc.vector.tensor_add(out=acc_q[e], in0=acc_q[e], in1=tmp)
                            if e == 0 and ib == 1:
                                # nk2 reduce + rk = 1/sqrt(nk2) in one ACT table
                                # op (the +1e-5 in the reference is a 1.6e-7
                                # relative effect at these norms; DVE reciprocal
                                # on [1, d] is a 6.5us single-partition op that
                                # head-of-line-blocks the DVE queue)
                                for ee in range(n_ef):
                                    nc.tensor.matmul(nk2_ps[ee], lhsT=onesb,
                                                     rhs=acc_k[ee],
                                                     start=True, stop=True)
                                    nc.scalar.activation(
                                        out=nk[0:1, ts(ee, ef)], in_=nk2_ps[ee],
                                        func=AF.Abs_reciprocal_sqrt)
                        if e == 0:
                            # rk transpose to per-partition column (tiny PE matmuls
                            # interleaved between Uq groups)
                            for db in range(n_dblk):
                                pm = mid_ps.tile([P, ef], F32, name="pm", tag="midps")
                                nc.tensor.matmul(pm[:, 0:1], lhsT=nk[0:1, ts(db, P)],
                                                 rhs=one11, start=True, stop=True)
                                nc.vector.tensor_copy(out=rk_col[:, db:db + 1],
                                                      in_=pm[:, 0:1])
                            nc.vector.tensor_scalar_mul(out=rk625, in0=rk_col,
                                                        scalar1=0.625)
                            nc.vector.tensor_scalar_mul(out=rk375, in0=rk_col,
                                                        scalar1=0.375)
                    # G = Tk^T @ WqT ; A' = rk * (0.625 G + 0.375 |G|) from PSUM.
                    # The trailing nq2 ones-matmul, the rq chain and the first
                    # VRVI chunk all interleave with the G groups so the PE
                    # never idles across the mid->pass2 transition.
                    g_groups = [(eb, e) for eb in range(n_dblk) for e in range(n_ef)]
                    for gi, (eb, e) in enumerate(g_groups):
                        pg = mid_ps.tile([P, ef], F32, name="pg", tag="midps")
                        for db in range(n_dblk):
                            nc.tensor.matmul(pg, lhsT=tkb_tiles[db][:, ts(eb, P)],
                                             rhs=w_tiles["wq"][db][:, ts(e, ef)],
                                             start=(db == 0), stop=(db == n_dblk - 1))
                        if gi == 0:
                            # nq2 banks reuse nk2's only now, after the nk
                            # Abs_reciprocal_sqrt reads were issued above
                            for ee in range(n_ef):
                                nq2_ps[ee] = nrm_ps.tile([1, ef], F32,
                                                         name=f"nq2{ee}",
                                                         tag=f"nrm{ee}")
                                nc.tensor.matmul(nq2_ps[ee], lhsT=onesb,
                                                 rhs=acc_q[ee],
                                                 start=True, stop=True)
                        if gi == 1:
                            # rq = 1/sqrt(nq2) on ACT under the G matmuls
                            for ee in range(n_ef):
                                nc.scalar.activation(out=nq[0:1, ts(ee, ef)],
                                                     in_=nq2_ps[ee],
                                                     func=AF.Abs_reciprocal_sqrt)
                        if gi == 3:
                            for ee in range(n_ef):
                                pb = mid_ps.tile([P, ef], F32, name="pb", tag="midps")
                                nc.tensor.matmul(pb, lhsT=ones_row,
                                                 rhs=nq[0:1, ts(ee, ef)],
                                                 start=True, stop=True)
                                nc.vector.tensor_copy(out=rq_bc[:, ts(ee, ef)], in_=pb)
                        tabs = wtmp_pool.tile([P, ef], F32, name="tabs", tag="wtmp")
                        nc.scalar.activation(out=tabs, in_=pg, func=AF.Abs,
                                             scale=rk375[:, eb:eb + 1])
                        nc.vector.scalar_tensor_tensor(
                            out=a_tiles[eb][:, ts(e, ef)], in0=pg,
                            scalar=rk625[:, eb:eb + 1], in1=tabs,
                            op0=ALU.mult, op1=ALU.add)

                    def issue_out(c, vts, ps_pool):
                        for s in range(n_sub):
                            for e in range(n_ef):
                                po = ps_pool.tile([P, ef], F32, name="po", tag="midps")
                                for eb in range(n_dblk):
                                    nc.tensor.matmul(po, lhsT=vts[eb][:, ts(s, P)],
                                                     rhs=a_tiles[eb][:, ts(e, ef)],
                                                     start=(eb == 0),
                                                     stop=(eb == n_dblk - 1))
                                ot = osb_pool.tile([P, ef], BF16, name="ot", tag="osb")
                                nc.vector.tensor_mul(out=ot, in0=po,
                                                     in1=rq_bc[:, ts(e, ef)])
                                nc.sync.dma_start(
                                    out=out_d[c * NC + s * P:c * NC + (s + 1) * P,
                                              ts(e, ef)],
                                    in_=ot)

                    # first two VRVI chunks AND the first out phase run out of
                    # the mid psum pool (same tile shape): every later pool-
                    # boundary bank reuse is then separated from its previous
                    # consumer by a full VRVI or out phase of PE work, so the
                    # transition never waits on a trailing gate chain
                    vts_fifo = [issue_vrvi_pool(0, xts_cache.pop(0), mid_ps),
                                issue_vrvi_pool(1, xts_cache.pop(1), mid_ps)]
                    xts_cache[2] = dma_xt(2)
                    issue_out(0, vts_fifo[0], mid_ps)

                # ---------------- Pass 2: VRVI / out pipeline ----------------
                with tc.tile_pool(name="vrvi_ps", bufs=4, space="PSUM") as vrvi_ps, \
                     tc.tile_pool(name="out_ps", bufs=4, space="PSUM") as out_ps:

                    def issue_vrvi(c, xts):
                        return issue_vrvi_pool(c, xts, vrvi_ps)

                    for c in range(1, n_chunks):
                        if c + 1 < n_chunks:
                            vts_fifo.append(issue_vrvi(c + 1, xts_cache.pop(c + 1)))
                        if c + 2 < n_chunks:
                            xts_cache[c + 2] = dma_xt(c + 2)
                        issue_out(c, vts_fifo[c], out_ps)
    nc.compile()
    return nc


_PROGRAM_CACHE = {}


def _get_program(n, d):
    key = (n, d)
    if key not in _PROGRAM_CACHE:
        _PROGRAM_CACHE[key] = build_program(n, d)
    return _PROGRAM_CACHE[key]


def _numpy_reference(x, Wvr, bvr, Wvi, bvi, Wk, bk, Wq, bq):
    """Slow fp32 fallback (never expected to run: biases are zeros)."""
    out = np.empty_like(x)
    for b in range(x.shape[0]):
        xb = x[b].astype(np.float64)
        vr = xb @ Wvr.T.astype(np.float64) + bvr
        vi = xb @ Wvi.T.astype(np.float64) + bvi
        v = vr * np.tanh(np.logaddexp(0.0, vi))
        k = xb @ Wk.T.astype(np.float64) + bk
        q = xb @ Wq.T.astype(np.float64) + bq
        kn = k / (np.linalg.norm(k, axis=0, keepdims=True) + 1e-5)
        qn = q / (np.linalg.norm(q, axis=0, keepdims=True) + 1e-5)
        g = kn.T @ qn
        a = 0.625 * g + 0.375 * np.abs(g)
        out[b] = (v @ a).astype(np.float32)
    return out


def kernel(_run_kwargs=None, **inputs):
    run_kwargs = _run_kwargs or {}
    x = np.asarray(inputs["x"], dtype=np.float32)
    Wvr = np.asarray(inputs["Wvr"], dtype=np.float32)
    Wvi = np.asarray(inputs["Wvi"], dtype=np.float32)
    Wk = np.asarray(inputs["Wk"], dtype=np.float32)
    Wq = np.asarray(inputs["Wq"], dtype=np.float32)
    bvr, bvi = np.asarray(inputs["bvr"]), np.asarray(inputs["bvi"])
    bk, bq = np.asarray(inputs["bk"]), np.asarray(inputs["bq"])

    if any(np.any(b != 0) for b in (bvr, bvi, bk, bq)):
        return _numpy_reference(x, Wvr, bvr, Wvi, bvi, Wk, bk, Wq, bq)

    b, n, d = x.shape
    assert b == B and n == N_FULL and d == D_FULL, (b, n, d)

    bf16 = ml_dtypes.bfloat16
    wvr_t = np.ascontiguousarray(Wvr.T).astype(bf16)
    wvi_t = np.ascontiguousarray(Wvi.T).astype(bf16)
    wk_t = np.ascontiguousarray(Wk.T).astype(bf16)
    wq_t = np.ascontiguousarray(Wq.T).astype(bf16)

    ident = np.eye(P, dtype=bf16)
    in_maps = []
    for i in range(N_CORES):
        in_maps.append({
            "xn": x[i].astype(bf16),
            "xt": np.ascontiguousarray(x[i].T).astype(bf16),
            "wvr": wvr_t, "wvi": wvi_t, "wk": wk_t, "wq": wq_t,
            "ident": ident,
        })

    nc = _get_program(n, d)
    from concourse.bass_utils import run_bass_kernel_spmd
    res = run_bass_kernel_spmd(nc, in_maps, core_ids=list(range(N_CORES)), **run_kwargs)
    out = np.stack([np.asarray(res.results[i]["out"]).astype(np.float32)
                    for i in range(N_CORES)], axis=0)
    if run_kwargs:
        kernel.last_results = res
    return out


# revision 43
# speedup vs baseline: 1.7328x; 1.0002x over previous
"""Trainium2 Bass kernel for nn_GatedFeedForward (gated feed-forward with
feature attention).

Reference computation per batch b (B=8, N=4096, D=1024):
    VR = x @ Wvr.T ; VI = x @ Wvi.T            (biases are zero)
    V  = VR * tanh(softplus(VI))
    K  = x @ Wk.T  ; Q  = x @ Wq.T
    Kn = K / (||K||_col + 1e-5) ; Qn = Q / (||Q||_col + 1e-5)   (norm over N)
    A  = smu(Kn.T @ Qn)     # == leaky-relu slope 0.25 == 0.625x + 0.375|x|
    out = V @ A
Sharding: pure data-parallel over batch — one batch per NeuronCore.

Key algebraic restructure: with S = X^T X (D x D, one N-contraction),
    K^T Q        = WkT^T S WqT          (WkT = Wk.T, [in,out])
    ||K_d||^2    = colsum(WkT * (S WkT))
    ||Q_e||^2    = colsum(WqT * (S WqT))
so the K/Q path costs one N*D^2 matmul (S) plus three D^3 matmuls
(Tk = S WkT, G = Tk^T WqT, Uq = S WqT) instead of two N*D^2 (K, Q) plus
one N*D^2 (K^T Q): 2.4x less PE time on that path. leaky's positive
homogeneity folds rk into A's rows and rq into the output tiles.

The gate tanh(softplus(x)) is evaluated as c0 + c1*tanh(a1 x + b1)
+ c2*tanh(a2 x + b2) (max abs err 3.9e-3): both ops hit the resident
tanh activation table — no table switches, no slow DVE reciprocal.

Schedule per core (all matmuls bf16 with fp32 PSUM):
  Pass 1: S accumulated directly in PSUM across all 8 sequence chunks,
      in two column-half phases of 8 banks each; cast to bf16 at phase end.
  Mid:    Tk (+nk2 via ones-matmuls), Uq (+nq2), G -> A' from PSUM;
      norm scalars overlap the Uq matmuls on ACT/DVE.
  Pass 2: VRVI(c) / out(c) software-pipelined with lag 1 so the PE never
      waits on the gate chain. Output stored bf16, upcast on host.
"""

import numpy as np
import ml_dtypes

import concourse.bass as bass
import concourse.tile as tile
from concourse import bacc, mybir
from concourse.bass import ts

F32 = mybir.dt.float32
BF16 = mybir.dt.bfloat16
AF = mybir.ActivationFunctionType
ALU = mybir.AluOpType

B = 8
N_FULL = 4096
D_FULL = 1024
N_CORES = 8

P = 128  # SBUF partitions
NC = 512  # sequence chunk
EF = 512  # free-dim span per matmul / psum bank

# tanh-sum fit of tanh(softplus(x)), max |err| 3.9e-3 on [-12, 12]
GC0, GC1, GA1, GB1 = 0.50022747, 0.32785149, 0.8261997, -0.02962021
GC2, GA2, GB2 = 0.17216236, 0.57575332, 0.75023909


def build_program(n=N_FULL, d=D_FULL):
    """Build the single-core SPMD Bass program for one [n, d] batch."""
    assert n % NC == 0 and d % P == 0
    n_chunks = n // NC
    n_sub = NC // P  # 128-row subtiles per chunk
    n_dblk = d // P  # feature blocks
    ef = min(EF, d)
    n_ef = d // ef

    nc = bacc.Bacc("TRN2", target_bir_lowering=False, debug=False,
                   num_devices=N_CORES)
    ident = nc.dram_tensor("ident", [P, P], BF16, kind="ExternalInput")
    xn = nc.dram_tensor("xn", [n, d], BF16, kind="ExternalInput")
    xt = nc.dram_tensor("xt", [d, n], BF16, kind="ExternalInput")
    wvr = nc.dram_tensor("wvr", [d, d], BF16, kind="ExternalInput")
    wvi = nc.dram_tensor("wvi", [d, d], BF16, kind="ExternalInput")
    wk = nc.dram_tensor("wk", [d, d], BF16, kind="ExternalInput")
    wq = nc.dram_tensor("wq", [d, d], BF16, kind="ExternalInput")
    out_d = nc.dram_tensor("out", [n, d], BF16, kind="ExternalOutput")

    with tile.TileContext(nc) as tc:
        with tc.tile_pool(name="const", bufs=1) as const_pool, \
             tc.tile_pool(name="w", bufs=1) as w_pool, \
             tc.tile_pool(name="sb", bufs=1) as sb_pool, \
             tc.tile_pool(name="tkb", bufs=1) as tkb_pool, \
             tc.tile_pool(name="post", bufs=1) as post_pool, \
             tc.tile_pool(name="apost", bufs=1) as ap_pool:
            ones32 = const_pool.tile([P, 1], F32, name="ones32", tag="ones32")
            nc.vector.memset(ones32, 1.0)
            # bf16 ones for the norm partition-reduce matmuls: fp32 moving
            # operands stream at half rate and add PE dtype-mode switches
            onesb = const_pool.tile([P, 1], BF16, name="onesb", tag="onesb")
            nc.vector.memset(onesb, 1.0)
            one11 = const_pool.tile([1, 1], F32, name="one11", tag="one11")
            nc.vector.memset(one11, 1.0)
            ones_row = const_pool.tile([1, P], F32, name="ones_row", tag="onesr")
            nc.vector.memset(ones_row, 1.0)
            gb1 = const_pool.tile([P, 1], F32, name="gb1", tag="gb1")
            nc.vector.memset(gb1, GB1)
            gb2 = const_pool.tile([P, 1], F32, name="gb2", tag="gb2")
            nc.vector.memset(gb2, GB2)
            ident_sb = const_pool.tile([P, P], BF16, name="ident", tag="ident")

            w_tiles = {}
            for wname, wdram in (("wk", wk), ("wq", wq), ("wvr", wvr), ("wvi", wvi)):
                w_tiles[wname] = [
                    w_pool.tile([P, d], BF16, name=f"{wname}{db}", tag=f"{wname}{db}")
                    for db in range(n_dblk)
                ]

            sb_tiles = [sb_pool.tile([P, d], BF16, name=f"s{ib}", tag=f"s{ib}")
                        for ib in range(n_dblk)]
            tkb_tiles = [tkb_pool.tile([P, d], BF16, name=f"tk{ib}", tag=f"tk{ib}")
                         for ib in range(n_dblk)]
            a_tiles = [ap_pool.tile([P, d], BF16, name=f"a{db}", tag=f"a{db}")
                       for db in range(n_dblk)]

            # ---------------- Pass 1: S = X^T X, PSUM-resident ----------------
            with tc.tile_pool(name="xn1", bufs=3) as xn_pool, \
                 tc.tile_pool(name="s_ps", bufs=1, space="PSUM") as s_ps:

                # the first chunk-sets ride two DMA queues (SP + ACT are the
                # hwdge initiators): at kernel start the serial per-queue
                # transfer stream is what paces the PE
                dma_split_left = [3]

                def dma_xn(e, c):
                    tiles = []
                    split = dma_split_left[0] > 0
                    if split:
                        dma_split_left[0] -= 1
                    for s in range(n_sub):
                        t = xn_pool.tile([P, d], BF16, name=f"xn{s}", tag=f"xn{s}")
                        eng = nc.scalar if (split and s % 2) else nc.sync
                        eng.dma_start(out=t, in_=xn[c * NC + s * P:c * NC + (s + 1) * P, :])
                        tiles.append(t)
                    return tiles

                # S is symmetric: compute only blocks with 128*ib <= 512*e+511
                # (the right half for all rows, then the upper-left quadrant)
                # and reconstruct the lower-left quadrant by PE transposes.
                # Right-half phase (8 psum banks) runs first so the transposes
                # and the 4-bank upper-left phase overlap cleanly.
                ib_set = {e: list(range(min(n_dblk, (ef * (e + 1)) // P)))
                          for e in range(n_ef)}
                seq = [(e, c) for e in sorted(range(n_ef), reverse=True)
                       for c in range(n_chunks)]
                xn_cache = {seq[0]: dma_xn(*seq[0])}
                # wk/wq stream during pass 1; wvr/wvi are issued at mid start
                wseq = [("wk", wk), ("wq", wq)]
                for idx, (e, c) in enumerate(seq):
                    # prefetch two chunk-sets ahead; weight DMAs interleave half
                    # a tensor per chunk so they never block the x stream
                    for j in (idx + 1, idx + 2):
                        if j < len(seq) and seq[j] not in xn_cache:
                            xn_cache[seq[j]] = dma_xn(*seq[j])
                    if idx == 1:
                        nc.sync.dma_start(out=ident_sb, in_=ident[:, :])
                    if 2 <= idx <= 2 * len(wseq) + 1:
                        wname, wdram = wseq[(idx - 2) // 2]
                        half = (idx - 2) % 2
                        for db in range(half * n_dblk // 2, (half + 1) * n_dblk // 2):
                            nc.sync.dma_start(out=w_tiles[wname][db],
                                              in_=wdram[ts(db, P), :])
                    if c == 0:
                        ps_list = {ib: s_ps.tile([P, ef], F32, name=f"sps{ib}",
                                                 tag=f"sps{ib}")
                                   for ib in ib_set[e]}
                    xns = xn_cache.pop((e, c))
                    last = c == n_chunks - 1
                    # last chunk runs ib-outer so each accumulator group stops
                    # early and its cast drains under the remaining matmuls
                    order = ([(s, ib) for ib in ib_set[e] for s in range(n_sub)]
                             if last else
                             [(s, ib) for s in range(n_sub) for ib in ib_set[e]])
                    for s, ib in order:
                        nc.tensor.matmul(ps_list[ib], lhsT=xns[s][:, ts(ib, P)],
                                         rhs=xns[s][:, ts(e, ef)],
                                         start=(c == 0 and s == 0),
                                         stop=(last and s == n_sub - 1))
                        if last and s == n_sub - 1:
                            # alternate engines: halves the serial cast chain
                            if ib % 2 == 0:
                                nc.vector.tensor_copy(out=sb_tiles[ib][:, ts(e, ef)],
                                                      in_=ps_list[ib])
                            else:
                                nc.scalar.activation(out=sb_tiles[ib][:, ts(e, ef)],
                                                     in_=ps_list[ib], func=AF.Copy)

            # ---------------- pass-2 SBUF pools (open early for prefetch) ------
            nk = post_pool.tile([1, d], F32, name="nk", tag="nk")
            nq = post_pool.tile([1, d], F32, name="nq", tag="nq")
            rk_col = post_pool.tile([P, n_dblk], F32, name="rk_col", tag="rk_col")
            rq_bc = post_pool.tile([P, d], F32, name="rq_bc", tag="rq_bc")
            rk625 = post_pool.tile([P, n_dblk], F32, name="rk625", tag="rk625")
            rk375 = post_pool.tile([P, n_dblk], F32, name="rk375", tag="rk375")

            with tc.tile_pool(name="xt2", bufs=2) as xt2_pool, \
                 tc.tile_pool(name="vt", bufs=2) as vt_pool, \
                 tc.tile_pool(name="gate", bufs=2) as gate_pool, \
                 tc.tile_pool(name="wtmp", bufs=4) as wtmp_pool, \
                 tc.tile_pool(name="osb", bufs=3) as osb_pool:

                def dma_xt(c):
                    tiles = []
                    for db in range(n_dblk):
                        t = xt2_pool.tile([P, NC], BF16, name=f"x2_{db}", tag=f"x2_{db}")
                        nc.sync.dma_start(out=t, in_=xt[ts(db, P), ts(c, NC)])
                        tiles.append(t)
                    return tiles

                xts_cache = {0: dma_xt(0), 1: dma_xt(1)}

                def issue_vrvi_pool(c, xts, ps_pool):
                    vts = []
                    for eb in range(n_dblk):
                        pvr = ps_pool.tile([P, NC], F32, name="pvr", tag="midps")
                        pvi = ps_pool.tile([P, NC], F32, name="pvi", tag="midps")
                        for db in range(n_dblk):
                            st = dict(start=(db == 0), stop=(db == n_dblk - 1))
                            nc.tensor.matmul(pvr, lhsT=w_tiles["wvr"][db][:, ts(eb, P)],
                                             rhs=xts[db], **st)
                            nc.tensor.matmul(pvi, lhsT=w_tiles["wvi"][db][:, ts(eb, P)],
                                             rhs=xts[db], **st)
                        # gate = c0 + c1*tanh(a1 x + b1) + c2*tanh(a2 x + b2)
                        t1 = gate_pool.tile([P, NC], F32, name="t1", tag="t1")
                        nc.scalar.activation(out=t1, in_=pvi, func=AF.Tanh,
                                             scale=GA1, bias=gb1)
                        t2 = gate_pool.tile([P, NC], F32, name="t2", tag="t2")
                        nc.scalar.activation(out=t2, in_=pvi, func=AF.Tanh,
                                             scale=GA2, bias=gb2)
                        g2 = gate_pool.tile([P, NC], F32, name="g2", tag="g2")
                        nc.vector.scalar_tensor_tensor(out=g2, in0=t2, scalar=GC2 / GC1,
                                                       in1=t1, op0=ALU.mult, op1=ALU.add)
                        g3 = gate_pool.tile([P, NC], F32, name="g3", tag="g3")
                        nc.vector.tensor_scalar(out=g3, in0=g2, scalar1=GC1,
                                                scalar2=GC0, op0=ALU.mult, op1=ALU.add)
                        vt = vt_pool.tile([P, NC], BF16, name=f"vt{eb}", tag=f"vt{eb}")
                        nc.vector.tensor_mul(out=vt, in0=g3, in1=pvr)
                        vts.append(vt)
                    return vts

                # ---------------- Mid: Tk, Uq, G, norms, A' ----------------
                with tc.tile_pool(name="mid_ps", bufs=5, space="PSUM") as mid_ps, \
                     tc.tile_pool(name="tp_ps", bufs=1, space="PSUM") as tp_ps, \
                     tc.tile_pool(name="nrm_ps", bufs=1, space="PSUM") as nrm_ps:

                    # wvr/wvi stream in under the Tk matmuls; first needed by
                    # VRVI(0) at the end of the mid phase
                    for wname, wdram in (("wvr", wvr), ("wvi", wvi)):
                        for db in range(n_dblk):
                            nc.sync.dma_start(out=w_tiles[wname][db],
                                              in_=wdram[ts(db, P), :])

                    # lower-left S blocks = transposes of the computed
                    # upper-right ones (S is symmetric, values bit-identical)
                    pairs = [(n_dblk // 2 + i, jb) for jb in range(n_dblk // 2)
                             for i in range(n_dblk // 2)]
                    for g in range(0, len(pairs), 4):
                        ptp = tp_ps.tile([P, ef], BF16, name="tps", tag="tps")
                        for k, (ibp, jb) in enumerate(pairs[g:g + 4]):
                            nc.tensor.transpose(out=ptp[:, ts(k, P)],
                                                in_=sb_tiles[jb][:, ts(ibp, P)],
                                                identity=ident_sb)
                        for k, (ibp, jb) in enumerate(pairs[g:g + 4]):
                            nc.vector.tensor_copy(out=sb_tiles[ibp][:, ts(jb, P)],
                                                  in_=ptp[:, ts(k, P)])

                    # Tk = S @ WkT ; nk2 = colsum(WkT * Tk).
                    # The W*T products accumulate on the DVE in bf16 (partition
                    # sums commute across ib blocks), so the partition-reduce is
                    # ONE ones-matmul per span instead of one per group — the
                    # per-group ones-matmuls cost ~0.4us each in stream time
                    # plus un-hidden LDWEIGHTS bubbles.
                    acc_k = {e: post_pool.tile([P, ef], BF16, name=f"acck{e}",
                                               tag=f"acck{e}") for e in range(n_ef)}
                    acc_q = {e: post_pool.tile([P, ef], BF16, name=f"accq{e}",
                                               tag=f"accq{e}") for e in range(n_ef)}
                    nk2_ps = {e: nrm_ps.tile([1, ef], F32, name=f"nk2{e}", tag=f"nrm{e}")
                              for e in range(n_ef)}
                    # ib-outer so the first 8 groups touch only the phase-A
                    # halves of S — the phase-B casts drain underneath them
                    for ib in range(n_dblk):
                        for e in range(n_ef):
                            pt = mid_ps.tile([P, ef], F32, name="pt", tag="midps")
                            for jb in range(n_dblk):
                                nc.tensor.matmul(pt, lhsT=sb_tiles[jb][:, ts(ib, P)],
                                                 rhs=w_tiles["wk"][jb][:, ts(e, ef)],
                                                 start=(jb == 0), stop=(jb == n_dblk - 1))
                            # cast on ACT (idle here) to keep the DVE under the
                            # matmul pace with the accumulate added
                            nc.scalar.activation(out=tkb_tiles[ib][:, ts(e, ef)],
                                                 in_=pt, func=AF.Copy)
                            tmp = wtmp_pool.tile([P, ef], BF16, name="tmp", tag="wtmpb")
                            nc.vector.tensor_mul(out=tmp, in0=pt,
                                                 in1=w_tiles["wk"][ib][:, ts(e, ef)])
                            if ib == 0:
                                nc.vector.tensor_copy(out=acc_k[e], in_=tmp)
                            else:
                                nc.vector.tensor_add(out=acc_k[e], in0=acc_k[e], in1=tmp)

                    # Uq = S @ WqT ; nq2 = colsum(WqT * Uq)
                    nq2_ps = {}
                    for e in range(n_ef):
                        for ib in range(n_dblk):
                            pt = mid_ps.tile([P, ef], F32, name="pu", tag="midps")
                            for jb in range(n_dblk):
                                nc.tensor.matmul(pt, lhsT=sb_tiles[jb][:, ts(ib, P)],
                                                 rhs=w_tiles["wq"][jb][:, ts(e, ef)],
                                                 start=(jb == 0), stop=(jb == n_dblk - 1))
                            tmp = wtmp_pool.tile([P, ef], BF16, name="tmq", tag="wtmpb")
                            nc.vector.tensor_mul(out=tmp, in0=pt,
                                                 in1=w_tiles["wq"][ib][:, ts(e, ef)])
                            if ib == 0:
                                nc.vector.tensor_copy(out=acc_q[e], in_=tmp)
                            else:
                                nc.vector.tensor_add(out=acc_q[e], in0=acc_q[e], in1=tmp)
                            if e == 0 and ib == 1:
                                # nk2 reduce + rk = 1/sqrt(nk2) in one ACT table
                                # op (the +1e-5 in the reference is a 1.6e-7
                                # relative effect at these norms; DVE reciprocal
                                # on [1, d] is a 6.5us single-partition op that
                                # head-of-line-blocks the DVE queue)
                                for ee in range(n_ef):
                                    nc.tensor.matmul(nk2_ps[ee], lhsT=onesb,
                                                     rhs=acc_k[ee],
                                                     start=True, stop=True)
                                    nc.scalar.activation(
                                        out=nk[0:1, ts(ee, ef)], in_=nk2_ps[ee],
                                        func=AF.Abs_reciprocal_sqrt)
                        if e == 0:
                            # rk transpose to per-partition column (tiny PE matmuls
                            # interleaved between Uq groups)
                            for db in range(n_dblk):
                                pm = mid_ps.tile([P, ef], F32, name="pm", tag="midps")
                                nc.tensor.matmul(pm[:, 0:1], lhsT=nk[0:1, ts(db, P)],
                                                 rhs=one11, start=True, stop=True)
                                nc.vector.tensor_copy(out=rk_col[:, db:db + 1],
                                                      in_=pm[:, 0:1])
                            nc.vector.tensor_scalar_mul(out=rk625, in0=rk_col,
                                                        scalar1=0.625)
                            nc.vector.tensor_scalar_mul(out=rk375, in0=rk_col,
                                                        scalar1=0.375)
                    # G = Tk^T @ WqT ; A' = rk * (0.625 G + 0.375 |G|) from PSUM.
                    # The trailing nq2 ones-matmul, the rq chain and the first
                    # VRVI chunk all interleave with the G groups so the PE
                    # never idles across the mid->pass2 transition.
                    g_groups = [(eb, e) for eb in range(n_dblk) for e in range(n_ef)]
                    for gi, (eb, e) in enumerate(g_groups):
                        pg = mid_ps.tile([P, ef], F32, name="pg", tag="midps")
                        for db in range(n_dblk):
                            nc.tensor.matmul(pg, lhsT=tkb_tiles[db][:, ts(eb, P)],
                                             rhs=w_tiles["wq"][db][:, ts(e, ef)],
                                             start=(db == 0), stop=(db == n_dblk - 1))
                        if gi == 0:
                            # nq2 banks reuse nk2's only now, after the nk
                            # Abs_reciprocal_sqrt reads were issued above
                            for ee in range(n_ef):
                                nq2_ps[ee] = nrm_ps.tile([1, ef], F32,
                                                         name=f"nq2{ee}",
                                                         tag=f"nrm{ee}")
                                nc.tensor.matmul(nq2_ps[ee], lhsT=onesb,
                                                 rhs=acc_q[ee],
                                                 start=True, stop=True)
                        if gi == 1:
                            # rq = 1/sqrt(nq2) on ACT under the G matmuls
                            for ee in range(n_ef):
                                nc.scalar.activation(out=nq[0:1, ts(ee, ef)],
                                                     in_=nq2_ps[ee],
                                                     func=AF.Abs_reciprocal_sqrt)
                        if gi == 3:
                            for ee in range(n_ef):
                                pb = mid_ps.tile([P, ef], F32, name="pb", tag="midps")
                                nc.tensor.matmul(pb, lhsT=ones_row,
                                                 rhs=nq[0:1, ts(ee, ef)],
                                                 start=True, stop=True)
                                nc.vector.tensor_copy(out=rq_bc[:, ts(ee, ef)], in_=pb)
                        tabs = wtmp_pool.tile([P, ef], F32, name="tabs", tag="wtmp")
                        nc.scalar.activation(out=tabs, in_=pg, func=AF.Abs,
                                             scale=rk375[:, eb:eb + 1])
                        nc.vector.scalar_tensor_tensor(
                            out=a_tiles[eb][:, ts(e, ef)], in0=pg,
                            scalar=rk625[:, eb:eb + 1], in1=tabs,
                            op0=ALU.mult, op1=ALU.add)

                    def issue_out(c, vts, ps_pool):
                        for s in range(n_sub):
                            for e in range(n_ef):
                                po = ps_pool.tile([P, ef], F32, name="po", tag="midps")
                                for eb in range(n_dblk):
                                    nc.tensor.matmul(po, lhsT=vts[eb][:, ts(s, P)],
                                                     rhs=a_tiles[eb][:, ts(e, ef)],
                                                     start=(eb == 0),
                                                     stop=(eb == n_dblk - 1))
                                ot = osb_pool.tile([P, ef], BF16, name="ot", tag="osb")
                                nc.vector.tensor_mul(out=ot, in0=po,
                                                     in1=rq_bc[:, ts(e, ef)])
                                nc.sync.dma_start(
                                    out=out_d[c * NC + s * P:c * NC + (s + 1) * P,
                                              ts(e, ef)],
                                    in_=ot)

                    # first two VRVI chunks AND the first out phase run out of
                    # the mid psum pool (same tile shape): every later pool-
                    # boundary bank reuse is then separated from its previous
                    # consumer by a full VRVI or out phase of PE work, so the
                    # transition never waits on a trailing gate chain
                    vts_fifo = [issue_vrvi_pool(0, xts_cache.pop(0), mid_ps),
                                issue_vrvi_pool(1, xts_cache.pop(1), mid_ps)]
                    xts_cache[2] = dma_xt(2)
                    issue_out(0, vts_fifo[0], mid_ps)

                # ---------------- Pass 2: VRVI / out pipeline ----------------
                with tc.tile_pool(name="vrvi_ps", bufs=4, space="PSUM") as vrvi_ps, \
                     tc.tile_pool(name="out_ps", bufs=4, space="PSUM") as out_ps:

                    def issue_vrvi(c, xts):
                        return issue_vrvi_pool(c, xts, vrvi_ps)

                    for c in range(1, n_chunks):
                        if c + 1 < n_chunks:
                            vts_fifo.append(issue_vrvi(c + 1, xts_cache.pop(c + 1)))
                        if c + 2 < n_chunks:
                            xts_cache[c + 2] = dma_xt(c + 2)
                        issue_out(c, vts_fifo[c], out_ps)
    nc.compile()
    return nc


_PROGRAM_CACHE = {}


def _get_program(n, d):
    key = (n, d)
    if key not in _PROGRAM_CACHE:
        _PROGRAM_CACHE[key] = build_program(n, d)
    return _PROGRAM_CACHE[key]


def _numpy_reference(x, Wvr, bvr, Wvi, bvi, Wk, bk, Wq, bq):
    """Slow fp32 fallback (never expected to run: biases are zeros)."""
    out = np.empty_like(x)
    for b in range(x.shape[0]):
        xb = x[b].astype(np.float64)
        vr = xb @ Wvr.T.astype(np.float64) + bvr
        vi = xb @ Wvi.T.astype(np.float64) + bvi
        v = vr * np.tanh(np.logaddexp(0.0, vi))
        k = xb @ Wk.T.astype(np.float64) + bk
        q = xb @ Wq.T.astype(np.float64) + bq
        kn = k / (np.linalg.norm(k, axis=0, keepdims=True) + 1e-5)
        qn = q / (np.linalg.norm(q, axis=0, keepdims=True) + 1e-5)
        g = kn.T @ qn
        a = 0.625 * g + 0.375 * np.abs(g)
        out[b] = (v @ a).astype(np.float32)
    return out


def kernel(_run_kwargs=None, **inputs):
    run_kwargs = _run_kwargs or {}
    x = np.asarray(inputs["x"], dtype=np.float32)
    Wvr = np.asarray(inputs["Wvr"], dtype=np.float32)
    Wvi = np.asarray(inputs["Wvi"], dtype=np.float32)
    Wk = np.asarray(inputs["Wk"], dtype=np.float32)
    Wq = np.asarray(inputs["Wq"], dtype=np.float32)
    bvr, bvi = np.asarray(inputs["bvr"]), np.asarray(inputs["bvi"])
    bk, bq = np.asarray(inputs["bk"]), np.asarray(inputs["bq"])

    if any(np.any(b != 0) for b in (bvr, bvi, bk, bq)):
        return _numpy_reference(x, Wvr, bvr, Wvi, bvi, Wk, bk, Wq, bq)

    b, n, d = x.shape
    assert b == B and n == N_FULL and d == D_FULL, (b, n, d)

    bf16 = ml_dtypes.bfloat16
    wvr_t = np.ascontiguousarray(Wvr.T).astype(bf16)
    wvi_t = np.ascontiguousarray(Wvi.T).astype(bf16)
    wk_t = np.ascontiguousarray(Wk.T).astype(bf16)
    wq_t = np.ascontiguousarray(Wq.T).astype(bf16)

    ident = np.eye(P, dtype=bf16)
    in_maps = []
    for i in range(N_CORES):
        in_maps.append({
            "xn": x[i].astype(bf16),
            "xt": np.ascontiguousarray(x[i].T).astype(bf16),
            "wvr": wvr_t, "wvi": wvi_t, "wk": wk_t, "wq": wq_t,
            "ident": ident,
        })

    nc = _get_program(n, d)
    from concourse.bass_utils import run_bass_kernel_spmd
    res = run_bass_kernel_spmd(nc, in_maps, core_ids=list(range(N_CORES)), **run_kwargs)
    out = np.stack([np.asarray(res.results[i]["out"]).astype(np.float32)
                    for i in range(N_CORES)], axis=0)
    if run_kwargs:
        kernel.last_results = res
    return out
